# revision 26
# baseline (speedup 1.0000x reference)
"""Trainium2 Bass kernel for residual-VQ autoencoder (nn_Autoencoder_45148696216751).

v3: encoder/decoder folding + DVE-minimal rescue.

Per core (data-parallel over tokens, 8 cores x 2048 tokens):
  stage-1 sweep runs directly on fp8(x) against cbE = codebook @ enc_w
  (host-folded), so the exact f32 encoder (z = x @ enc_w.T + b) overlaps
  under the sweep instead of gating it.
  2x VQ stage:
     scores[t,k] = lam*(r.c - |c|^2/2) via fp8 DoubleRow matmuls
     (4 data + 2 bias-partial DR matmuls per 1024-code superchunk)
     DVE: max8 -> m8buf (top-8 per superchunk, exact f32) and
          find_index8 -> idxbuf (within-superchunk positions)
     rescue per token tile: eps-perturbed m8 -> global max8 + find_index8
     gives the top-8 SLOTS; gpsimd gathers absolute code ids by slot;
     ONE batched indirect-DMA gather of 6 candidate rows [cb|bias];
     exact f32 rescore dots on gpsimd/scalar; winner picked by is_ge
     match; winner row gathered from cbx (residual update) and from
     cbD = codebook @ dec_w.T (host-folded decoder) for the output
     accumulation out[t] = cbD[w1] + cbD[w2] + dec_b. No decoder matmul,
     no qT transposes.
"""
import os, sys, types

os.environ.setdefault("NEURON_RT_RESET_CORES", "1")
sys.path.insert(0, '/opt/trn_rl_repo')
import numpy as np

import concourse.bass as bass
import concourse.tile as tile
from concourse import bacc, mybir
from concourse.bass_utils import run_bass_kernel_spmd
from concourse.masks import make_identity

f32 = mybir.dt.float32
fp8 = mybir.dt.float8e4
i32 = mybir.dt.int32
u16 = mybir.dt.uint16
ALU = mybir.AluOpType
DR = mybir.MatmulPerfMode.DoubleRow
AF = mybir.ActivationFunctionType

NCORES = 8
B, N, D = 4, 4096, 512
T = B * N                 # 16384 tokens
TL = T // NCORES          # 2048 tokens per core
K = 16384                 # codebook size
NT = TL // 128            # 16 token tiles per core
NJ = D // 128             # 4 contraction tiles
SC = 1024                 # superchunk (2 psum banks)
NSC = K // SC             # 16 superchunks
NR8 = 6                   # fp8 stream rows (4 cb + 2 bias-partial)
NUM_Q = 2
NCAND = 8                 # rescued candidates per token
PADW = 520                # cbx row: 512 cb + 1 bias + pad
LAM = 0.5                 # score scale so |lam*bias| fits fp8 range
QSCALE = 6.0              # pack: quantize m8 score to 1/6 grid
QOFF = 1252.0             # pack offset so quantized scores are >= 0
QMUL = 16384.0            # pack: id rides the low 14 bits


def _ensure_axon_hook():
    """Register the NTFF profile hook (missing antenv.axon_hooks shim)."""
    if "antenv.axon_hooks" in sys.modules:
        return
    mod = types.ModuleType("antenv.axon_hooks")
    _h = [None]
    mod.set_axon_ntff_profile_hook = lambda h: _h.__setitem__(0, h)
    mod.get_axon_ntff_profile_hook = lambda: _h[0]
    sys.modules["antenv.axon_hooks"] = mod
    try:
        import antenv
        antenv.axon_hooks = mod
        from trn_agent_boot.trn_boot import _ntff_profile_via_ctypes
        hook = _ntff_profile_via_ctypes('/opt/axon/libaxon_pjrt.so')
        if hook is not None:
            mod.set_axon_ntff_profile_hook(hook)
    except Exception:
        pass


def _build():
    nc = bacc.Bacc("TRN2", target_bir_lowering=False, debug=False,
                   num_devices=NCORES)

    xT_d = nc.dram_tensor("xT", [128, NJ, TL], f32, kind="ExternalInput")
    xT8_d = nc.dram_tensor("xT8", [128, NJ, TL], fp8, kind="ExternalInput")
    cbsE_d = nc.dram_tensor("cbsE", [128, NR8, K], fp8, kind="ExternalInput")
    cbs2_d = nc.dram_tensor("cbs2", [128, NR8, K], fp8, kind="ExternalInput")
    cbx_d = nc.dram_tensor("cbx", [K, PADW], f32, kind="ExternalInput")
    cbq_d = nc.dram_tensor("cbq", [K, D], f32, kind="ExternalInput")
    cbD_d = nc.dram_tensor("cbD", [K, D], f32, kind="ExternalInput")
    ewT_d = nc.dram_tensor("ewT", [128, NJ, D], f32, kind="ExternalInput")
    ebf_d = nc.dram_tensor("ebf", [128, D], f32, kind="ExternalInput")
    db_d = nc.dram_tensor("db", [128, D], f32, kind="ExternalInput")
    ones8_d = nc.dram_tensor("ones8", [128, 2, 128], fp8, kind="ExternalInput")
    out_d = nc.dram_tensor("out", [TL, D], f32, kind="ExternalOutput")

    from contextlib import ExitStack
    with tile.TileContext(nc) as tc, ExitStack() as ctx:
        big = ctx.enter_context(tc.tile_pool(name="big", bufs=1))
        xp = ctx.enter_context(tc.tile_pool(name="xp", bufs=1))
        cbp = ctx.enter_context(tc.tile_pool(name="cbp", bufs=3))
        q8p = ctx.enter_context(tc.tile_pool(name="q8p", bufs=2))
        qtp = ctx.enter_context(tc.tile_pool(name="qtp", bufs=3))
        drp = ctx.enter_context(tc.tile_pool(name="drp", bufs=3))
        outp = ctx.enter_context(tc.tile_pool(name="outp", bufs=2))
        scrp = ctx.enter_context(tc.tile_pool(name="scr", bufs=2))
        smallp = ctx.enter_context(tc.tile_pool(name="small", bufs=8))
        psc = ctx.enter_context(tc.tile_pool(name="psc", bufs=3, space="PSUM"))
        psm = ctx.enter_context(tc.tile_pool(name="psm", bufs=2, space="PSUM"))

        # ---- persistent tiles
        xT8 = big.tile([128, NJ, TL], fp8)     # stage-1 lhsT (fp8 of x)
        rT8 = big.tile([128, NJ, TL], fp8)     # stage-2 lhsT (fp8 residual)
        z_td = big.tile([128, NT, D], f32)     # exact residual, [t, d] layout
        qacc = big.tile([128, NT, D], f32)     # output accumulator
        ewT = big.tile([128, NJ, D], f32)
        ebf = big.tile([128, D], f32)
        db = big.tile([128, D], f32)
        ones8 = big.tile([128, 2, 128], fp8)
        ident = big.tile([128, 128], f32)
        m8buf = big.tile([128, NT, NSC * 8], f32)
        idxbuf = big.tile([128, NT, NSC * 8], u16)
        iota_off8 = big.tile([128, NSC * 8], f32)

        nc.sync.dma_start(xT8[:], xT8_d.ap())
        nc.sync.dma_start(ewT[:], ewT_d.ap())
        nc.sync.dma_start(ebf[:], ebf_d.ap())
        nc.sync.dma_start(db[:], db_d.ap())
        nc.sync.dma_start(ones8[:], ones8_d.ap())
        make_identity(nc, ident[:])
        nc.gpsimd.iota(iota_off8[:], pattern=[[SC, NSC], [0, 8]], base=0,
                       channel_multiplier=0,
                       allow_small_or_imprecise_dtypes=True)


        # ---- encoder (overlapped under stage-1 sweep): z_td = x @ ewT + eb
        HL = TL // 2
        xth = [None, None]

        def emit_enc_dma(half):
            xth[half] = xp.tile([128, NJ, HL], f32, tag="x", name="xth")
            nc.sync.dma_start(xth[half][:],
                              xT_d.ap()[:, :, half * HL:(half + 1) * HL])

        def emit_enc_tile(t):
            half, t2 = divmod(t, HL // 128)
            psz = psm.tile([128, 512], f32, tag="m")
            for j in range(NJ):
                nc.tensor.matmul(
                    psz[:], lhsT=xth[half][:, j, t2 * 128:(t2 + 1) * 128],
                    rhs=ewT[:, j, :],
                    start=(j == 0), stop=(j == NJ - 1))
            nc.scalar.copy(z_td[:, t, :], psz[:])
            nc.gpsimd.tensor_add(z_td[:, t, :], z_td[:, t, :], ebf[:])

        def emit_rt8(t):
            """rT8[:, :, t*128:(t+1)*128] = fp8(transpose(z_td[:, t, :]))."""
            tp = psm.tile([128, NJ, 128], f32, tag="m")
            for j in range(NJ):
                nc.tensor.transpose(tp[:, j, :],
                                    z_td[:, t, j * 128:(j + 1) * 128],
                                    ident[:])
            nc.scalar.copy(rT8[:, :, t * 128:(t + 1) * 128], tp[:])

        # ---- fp8 sweep + scans
        lhs = [xT8, rT8]
        cbs_d = [cbsE_d, cbs2_d]

        def emit_sweep_sc(s, tset, sc):
            cbt = cbp.tile([128, NR8, SC], fp8, tag="cbt")
            nc.sync.dma_start(cbt[:], cbs_d[s].ap()[:, :, sc * SC:(sc + 1) * SC])
            lh = lhs[s]
            for t in tset:
                tsl = slice(t * 128, (t + 1) * 128)
                ps = psc.tile([128, SC], f32, tag="sc")
                for h in range(SC // 512):
                    pslice = ps[:, h * 512:(h + 1) * 512]
                    cslice = slice(h * 512, (h + 1) * 512)
                    nc.tensor.matmul(
                        pslice, lhsT=lh[:, 0:2, tsl],
                        rhs=cbt[:, 0:2, cslice],
                        start=True, stop=False, perf_mode=DR)
                    nc.tensor.matmul(
                        pslice, lhsT=lh[:, 2:4, tsl],
                        rhs=cbt[:, 2:4, cslice],
                        start=False, stop=False, perf_mode=DR)
                    nc.tensor.matmul(
                        pslice, lhsT=ones8[:],
                        rhs=cbt[:, 4:6, cslice],
                        start=False, stop=True, perf_mode=DR)
                m8s = m8buf[:, t, sc * 8:sc * 8 + 8]
                nc.vector.max(out=m8s, in_=ps[:])
                nc.vector.max_index(out=idxbuf[:, t, sc * 8:sc * 8 + 8],
                                    in_max=m8s, in_values=ps[:])

        # ---- rescue pipeline (per token tile, per stage)
        comb = {}

        def emit_a(s, t):
            st = {}
            # absolute code ids
            idxf = smallp.tile([128, NSC * 8], f32, tag="idxf")
            nc.gpsimd.tensor_copy(idxf[:], idxbuf[:, t, :])
            nc.gpsimd.tensor_add(idxf[:], idxf[:], iota_off8[:])
            # pack (quantized score, id) into exact f32 integers:
            # pk = trunc_or_rne(m8*QSCALE + QOFF)*QMUL + id
            a1 = scrp.tile([128, NSC * 8], f32, tag="a1")
            nc.scalar.activation(a1[:], m8buf[:, t, :], AF.Copy,
                                 bias=QOFF, scale=QSCALE)
            q32 = scrp.tile([128, NSC * 8], i32, tag="q32")
            nc.scalar.copy(q32[:], a1[:])
            a2 = scrp.tile([128, NSC * 8], f32, tag="a2")
            nc.scalar.activation(a2[:], q32[:], AF.Copy, scale=QMUL)
            pk = scrp.tile([128, NSC * 8], f32, tag="pk")
            nc.gpsimd.tensor_add(pk[:], a2[:], idxf[:])
            # top-8 packed values carry their ids; decode low 14 bits
            g8p = smallp.tile([128, 8], f32, tag="g8")
            nc.vector.max(out=g8p[:], in_=pk[:])
            qaf = smallp.tile([128, 8], f32, tag="qaf")
            nc.vector.tensor_scalar(qaf[:], g8p[:], scalar1=1.0 / QMUL,
                                    scalar2=None, op0=ALU.mult)
            qa32 = smallp.tile([128, 8], i32, tag="qa32")
            nc.vector.tensor_copy(qa32[:], qaf[:])
            qab = smallp.tile([128, 8], f32, tag="qab")
            nc.vector.tensor_copy(qab[:], qa32[:])
            candf = smallp.tile([128, 8], f32, tag="cf")
            nc.vector.scalar_tensor_tensor(
                out=candf[:], in0=qab[:], scalar=-QMUL, in1=g8p[:],
                op0=ALU.mult, op1=ALU.add)
            # cast may round up: fix negative remainders (+QMUL)
            m1 = smallp.tile([128, 8], f32, tag="m1")
            nc.vector.tensor_scalar(m1[:], candf[:], scalar1=0.0,
                                    scalar2=QMUL, op0=ALU.is_lt,
                                    op1=ALU.mult)
            nc.vector.tensor_add(candf[:], candf[:], m1[:])
            cand32 = smallp.tile([128, NCAND], i32, tag="c32")
            nc.vector.tensor_copy(cand32[:], candf[:, 0:NCAND])
            q8 = q8p.tile([128, NCAND, PADW], f32, tag="q8")
            for k in range(NCAND):
                nc.gpsimd.indirect_dma_start(
                    out=q8[:, k, :], out_offset=None, in_=cbx_d.ap(),
                    in_offset=bass.IndirectOffsetOnAxis(
                        ap=cand32[:, k:k + 1], axis=0))
            st["candf"], st["q8"] = candf, q8
            comb[(s, t)] = st

        def emit_b(s, t):
            st = comb[(s, t)]
            candf, q8 = st["candf"], st["q8"]
            sc8 = smallp.tile([128, NCAND], f32, tag="sc8")
            for k in range(NCAND):
                junk2 = scrp.tile([128, D], f32, tag="j2")
                nc.gpsimd.tensor_mul(junk2[:], q8[:, k, 0:D],
                                     z_td[:, t, :])
                nc.scalar.activation(
                    out=junk2[:], in_=junk2[:], func=AF.Copy,
                    accum_out=sc8[:, k:k + 1])
            nc.vector.tensor_add(sc8[:], sc8[:],
                                 q8[:, :, D:D + 1].squeeze())
            gm = smallp.tile([128, 1], f32, tag="gm")
            nc.vector.tensor_reduce(gm[:], sc8[:],
                                    axis=mybir.AxisListType.X,
                                    op=ALU.max)
            junk8 = smallp.tile([128, NCAND], f32, tag="j8")
            wf = smallp.tile([128, 1], f32, tag="wf")
            nc.vector.scalar_tensor_tensor(
                out=junk8[:], in0=sc8[:], scalar=gm[:],
                in1=candf[:, 0:NCAND], op0=ALU.is_ge, op1=ALU.mult,
                accum_out=wf[:])
            wi32 = smallp.tile([128, 1], i32, tag="wi")
            nc.vector.tensor_copy(wi32[:], wf[:])
            drow = drp.tile([128, D], f32, tag="dr")
            nc.gpsimd.indirect_dma_start(
                out=drow[:], out_offset=None, in_=cbD_d.ap(),
                in_offset=bass.IndirectOffsetOnAxis(ap=wi32[:], axis=0))
            st["drow"] = drow
            if s == 0:
                qrow = qtp.tile([128, D], f32, tag="qt")
                nc.gpsimd.indirect_dma_start(
                    out=qrow[:], out_offset=None,
                    in_=cbq_d.ap(),
                    in_offset=bass.IndirectOffsetOnAxis(ap=wi32[:], axis=0))
                st["qrow"] = qrow

        def emit_c(s, t):
            st = comb.pop((s, t))
            drow = st["drow"]
            if s == 0:
                nc.gpsimd.tensor_sub(z_td[:, t, :], z_td[:, t, :],
                                     st["qrow"][:])
                emit_rt8(t)
                nc.gpsimd.tensor_add(qacc[:, t, :], drow[:], db[:])
            else:
                o_t = outp.tile([128, D], f32, tag="o")
                nc.gpsimd.tensor_add(o_t[:], qacc[:, t, :], drow[:])
                nc.sync.dma_start(out_d.ap()[t * 128:(t + 1) * 128, :], o_t[:])

        def pump(ps, tiles, r, coff=5):
            n = len(tiles)
            if 0 <= r < n:
                emit_a(ps, tiles[r])
            if 0 <= r - 2 < n:
                emit_b(ps, tiles[r - 2])
            if 0 <= r - coff < n:
                emit_c(ps, tiles[r - coff])

        GROUPS = [range(0, 8), range(8, 16)]
        emit_enc_dma(0)
        prev = None
        w = 0
        for s in range(NUM_Q):
            for tset in GROUPS:
                for sc in range(NSC):
                    emit_sweep_sc(s, tset, sc)
                    if prev is not None:
                        pump(prev[0], prev[1], sc)
                    if s == 0:
                        if w == 18:
                            emit_enc_dma(1)
                        if 3 <= w <= 17 and w % 2 == 1:
                            emit_enc_tile((w - 3) // 2)
                        elif 21 <= w <= 28:
                            emit_enc_tile(w - 13)
                        w += 1
                prev = (s, list(tset))

        # tail flush: tight pump cadence, gathers pipelined at depth 2
        ps_, tiles_ = prev
        for r in range(len(tiles_) + 5):
            pump(ps_, tiles_, r, coff=4)

    nc.compile()
    return nc


_CACHE = {}


def _get_nc():
    if "nc" not in _CACHE:
        _ensure_axon_hook()
        _CACHE["nc"] = _build()
    return _CACHE["nc"]


def _host_prep(x, enc_w, enc_b, codebook, dec_w, dec_b):
    import ml_dtypes
    f8 = ml_dtypes.float8_e4m3

    x = np.asarray(x, np.float32)
    enc_w = np.asarray(enc_w, np.float32)
    enc_b = np.asarray(enc_b, np.float32)
    cb = np.ascontiguousarray(np.asarray(codebook, np.float32))
    dec_w = np.asarray(dec_w, np.float32)
    dec_b = np.asarray(dec_b, np.float32)

    flat = x.reshape(T, D)
    csq = (cb.astype(np.float64) ** 2).sum(-1).astype(np.float32)
    bias = (-0.5 * csq).astype(np.float32)

    def fp8_stream(table, bias_vec):
        """[128, NR8, K] fp8: rows 0-3 = lam*table.T k-tiles, 4-5 = bias."""
        tT = np.ascontiguousarray(table.T)                # [D, K]
        cbs = np.zeros((128, NR8, K), np.float32)
        cbs[:, :NJ, :] = (LAM * tT).reshape(NJ, 128, K).transpose(1, 0, 2)
        rem = (LAM * bias_vec).astype(np.float32).copy()
        parts = []
        for _ in range(4):
            p = np.asarray(rem, f8).astype(np.float32)
            parts.append(p)
            rem = rem - p
        cbs[0, NJ, :] = parts[0]
        cbs[1, NJ, :] = parts[1]
        cbs[0, NJ + 1, :] = parts[2]
        cbs[1, NJ + 1, :] = parts[3]
        return np.asarray(cbs, f8)

    # stage-1 sweeps on x directly: z.c == x.(cb @ enc_w) + enc_b.c
    cbE = (cb @ enc_w).astype(np.float32)
    biasE = bias + (cb @ enc_b).astype(np.float32)
    cbsE = fp8_stream(cbE, biasE)
    cbs2 = fp8_stream(cb, bias)

    # rescore table [cb row | bias | pad]; decoder-folded gather table
    cbx = np.zeros((K, PADW), np.float32)
    cbx[:, :D] = cb
    cbx[:, D] = bias
    cbD = np.ascontiguousarray(cb @ dec_w.T.astype(np.float32))
    cbq = cb

    ewT = np.ascontiguousarray(
        enc_w.T.reshape(NJ, 128, D).transpose(1, 0, 2))   # [128, NJ, D]
    ebf = np.ascontiguousarray(np.broadcast_to(enc_b, (128, D)))
    dbf = np.ascontiguousarray(np.broadcast_to(dec_b, (128, D)))
    ones8 = np.ones((128, 2, 128), f8)

    common = {"cbsE": cbsE, "cbs2": cbs2, "cbx": cbx, "cbq": cbq,
              "cbD": cbD, "ewT": ewT, "ebf": ebf, "db": dbf, "ones8": ones8}

    in_maps = []
    for sh in range(NCORES):
        shard = flat[sh * TL:(sh + 1) * TL]               # [TL, D]
        xT = np.ascontiguousarray(
            shard.T.reshape(NJ, 128, TL).transpose(1, 0, 2))
        xT8 = np.asarray(xT, f8)
        in_maps.append({"xT": xT, "xT8": xT8, **common})
    return in_maps


def _run(inputs, trace=False):
    nc = _get_nc()
    in_maps = _host_prep(**inputs)
    res = run_bass_kernel_spmd(nc, in_maps, list(range(NCORES)), trace=trace)
    outs = [res.results[s]["out"] for s in range(NCORES)]
    full = np.concatenate(outs, axis=0).reshape(B, N, D)
    return full, res


def kernel(**inputs) -> np.ndarray:
    out, _ = _run(inputs, trace=False)
    return out


def kernel_traced(**inputs):
    out, res = _run(inputs, trace=True)
    return out, res


# revision 30
# speedup vs baseline: 1.0925x; 1.0925x over previous
"""Trainium2 Bass kernel for residual-VQ autoencoder (nn_Autoencoder_45148696216751).

v3: encoder/decoder folding + DVE-minimal rescue.

Per core (data-parallel over tokens, 8 cores x 2048 tokens):
  stage-1 sweep runs directly on fp8(x) against cbE = codebook @ enc_w
  (host-folded), so the exact f32 encoder (z = x @ enc_w.T + b) overlaps
  under the sweep instead of gating it.
  2x VQ stage:
     scores[t,k] = lam*(r.c - |c|^2/2) via fp8 DoubleRow matmuls
     (4 data + 2 bias-partial DR matmuls per 1024-code superchunk)
     DVE: max8 -> m8buf (top-8 per superchunk, exact f32) and
          find_index8 -> idxbuf (within-superchunk positions)
     rescue per token tile: eps-perturbed m8 -> global max8 + find_index8
     gives the top-8 SLOTS; gpsimd gathers absolute code ids by slot;
     ONE batched indirect-DMA gather of 6 candidate rows [cb|bias];
     exact f32 rescore dots on gpsimd/scalar; winner picked by is_ge
     match; winner row gathered from cbx (residual update) and from
     cbD = codebook @ dec_w.T (host-folded decoder) for the output
     accumulation out[t] = cbD[w1] + cbD[w2] + dec_b. No decoder matmul,
     no qT transposes.
"""
import os, sys, types

os.environ.setdefault("NEURON_RT_RESET_CORES", "1")
sys.path.insert(0, '/opt/trn_rl_repo')
import numpy as np

import concourse.bass as bass
import concourse.tile as tile
from concourse import bacc, mybir
from concourse.bass_utils import run_bass_kernel_spmd
from concourse.masks import make_identity

f32 = mybir.dt.float32
fp8 = mybir.dt.float8e4
i32 = mybir.dt.int32
u16 = mybir.dt.uint16
ALU = mybir.AluOpType
DR = mybir.MatmulPerfMode.DoubleRow
AF = mybir.ActivationFunctionType

NCORES = 8
B, N, D = 4, 4096, 512
T = B * N                 # 16384 tokens
TL = T // NCORES          # 2048 tokens per core
K = 16384                 # codebook size
NT = TL // 128            # 16 token tiles per core
NJ = D // 128             # 4 contraction tiles
SC = 1024                 # superchunk (2 psum banks)
NSC = K // SC             # 16 superchunks
NR8 = 6                   # fp8 stream rows (4 cb + 2 bias-partial)
NUM_Q = 2
NCAND = 6                 # rescued candidates per token
PADW = 520                # cbx row: 512 cb + 1 bias + pad
LAM = 0.5                 # score scale so |lam*bias| fits fp8 range
QSCALE = 6.0              # pack: quantize m8 score to 1/6 grid
QOFF = 1252.0             # pack offset so quantized scores are >= 0
QMUL = 16384.0            # pack: id rides the low 14 bits


def _ensure_axon_hook():
    """Register the NTFF profile hook (missing antenv.axon_hooks shim)."""
    if "antenv.axon_hooks" in sys.modules:
        return
    mod = types.ModuleType("antenv.axon_hooks")
    _h = [None]
    mod.set_axon_ntff_profile_hook = lambda h: _h.__setitem__(0, h)
    mod.get_axon_ntff_profile_hook = lambda: _h[0]
    sys.modules["antenv.axon_hooks"] = mod
    try:
        import antenv
        antenv.axon_hooks = mod
        from trn_agent_boot.trn_boot import _ntff_profile_via_ctypes
        hook = _ntff_profile_via_ctypes('/opt/axon/libaxon_pjrt.so')
        if hook is not None:
            mod.set_axon_ntff_profile_hook(hook)
    except Exception:
        pass


def _build():
    nc = bacc.Bacc("TRN2", target_bir_lowering=False, debug=False,
                   num_devices=NCORES)

    xT_d = nc.dram_tensor("xT", [128, NJ, TL], f32, kind="ExternalInput")
    xT8_d = nc.dram_tensor("xT8", [128, NJ, TL], fp8, kind="ExternalInput")
    cbsE_d = nc.dram_tensor("cbsE", [128, NR8, K], fp8, kind="ExternalInput")
    cbs2_d = nc.dram_tensor("cbs2", [128, NR8, K], fp8, kind="ExternalInput")
    cbx_d = nc.dram_tensor("cbx", [K, PADW], f32, kind="ExternalInput")
    cbq_d = nc.dram_tensor("cbq", [K, D], f32, kind="ExternalInput")
    cbD_d = nc.dram_tensor("cbD", [K, D], f32, kind="ExternalInput")
    ewT_d = nc.dram_tensor("ewT", [128, NJ, D], f32, kind="ExternalInput")
    ebf_d = nc.dram_tensor("ebf", [128, D], f32, kind="ExternalInput")
    db_d = nc.dram_tensor("db", [128, D], f32, kind="ExternalInput")
    ones8_d = nc.dram_tensor("ones8", [128, 2, 128], fp8, kind="ExternalInput")
    out_d = nc.dram_tensor("out", [TL, D], f32, kind="ExternalOutput")

    from contextlib import ExitStack
    with tile.TileContext(nc) as tc, ExitStack() as ctx:
        big = ctx.enter_context(tc.tile_pool(name="big", bufs=1))
        xp = ctx.enter_context(tc.tile_pool(name="xp", bufs=1))
        cbp = ctx.enter_context(tc.tile_pool(name="cbp", bufs=3))
        q8p = ctx.enter_context(tc.tile_pool(name="q8p", bufs=2))
        qtp = ctx.enter_context(tc.tile_pool(name="qtp", bufs=3))
        drp = ctx.enter_context(tc.tile_pool(name="drp", bufs=3))
        outp = ctx.enter_context(tc.tile_pool(name="outp", bufs=2))
        scrp = ctx.enter_context(tc.tile_pool(name="scr", bufs=2))
        smallp = ctx.enter_context(tc.tile_pool(name="small", bufs=8))
        psc = ctx.enter_context(tc.tile_pool(name="psc", bufs=3, space="PSUM"))
        psm = ctx.enter_context(tc.tile_pool(name="psm", bufs=2, space="PSUM"))

        # ---- persistent tiles
        xT8 = big.tile([128, NJ, TL], fp8)     # stage-1 lhsT (fp8 of x)
        rT8 = big.tile([128, NJ, TL], fp8)     # stage-2 lhsT (fp8 residual)
        z_td = big.tile([128, NT, D], f32)     # exact residual, [t, d] layout
        qacc = big.tile([128, NT, D], f32)     # output accumulator
        ewT = big.tile([128, NJ, D], f32)
        ebf = big.tile([128, D], f32)
        db = big.tile([128, D], f32)
        ones8 = big.tile([128, 2, 128], fp8)
        ident = big.tile([128, 128], f32)
        m8buf = big.tile([128, NT, NSC * 8], f32)
        idxbuf = big.tile([128, NT, NSC * 8], u16)
        iota_off8 = big.tile([128, NSC * 8], f32)

        nc.sync.dma_start(xT8[:], xT8_d.ap())
        nc.sync.dma_start(ewT[:], ewT_d.ap())
        nc.sync.dma_start(ebf[:], ebf_d.ap())
        nc.sync.dma_start(db[:], db_d.ap())
        nc.sync.dma_start(ones8[:], ones8_d.ap())
        make_identity(nc, ident[:])
        nc.gpsimd.iota(iota_off8[:], pattern=[[SC, NSC], [0, 8]], base=0,
                       channel_multiplier=0,
                       allow_small_or_imprecise_dtypes=True)


        # ---- encoder (overlapped under stage-1 sweep): z_td = x @ ewT + eb
        HL = TL // 2
        xth = [None, None]

        def emit_enc_dma(half):
            xth[half] = xp.tile([128, NJ, HL], f32, tag="x", name="xth")
            nc.sync.dma_start(xth[half][:],
                              xT_d.ap()[:, :, half * HL:(half + 1) * HL])

        def emit_enc_tile(t):
            half, t2 = divmod(t, HL // 128)
            psz = psm.tile([128, 512], f32, tag="m")
            for j in range(NJ):
                nc.tensor.matmul(
                    psz[:], lhsT=xth[half][:, j, t2 * 128:(t2 + 1) * 128],
                    rhs=ewT[:, j, :],
                    start=(j == 0), stop=(j == NJ - 1))
            nc.scalar.copy(z_td[:, t, :], psz[:])
            nc.gpsimd.tensor_add(z_td[:, t, :], z_td[:, t, :], ebf[:])

        def emit_rt8(t):
            """rT8[:, :, t*128:(t+1)*128] = fp8(transpose(z_td[:, t, :]))."""
            tp = psm.tile([128, NJ, 128], f32, tag="m")
            for j in range(NJ):
                nc.tensor.transpose(tp[:, j, :],
                                    z_td[:, t, j * 128:(j + 1) * 128],
                                    ident[:])
            nc.scalar.copy(rT8[:, :, t * 128:(t + 1) * 128], tp[:])

        # ---- fp8 sweep + scans
        lhs = [xT8, rT8]
        cbs_d = [cbsE_d, cbs2_d]

        def emit_sweep_sc(s, tset, sc):
            cbt = cbp.tile([128, NR8, SC], fp8, tag="cbt")
            nc.sync.dma_start(cbt[:], cbs_d[s].ap()[:, :, sc * SC:(sc + 1) * SC])
            lh = lhs[s]
            for t in tset:
                tsl = slice(t * 128, (t + 1) * 128)
                ps = psc.tile([128, SC], f32, tag="sc")
                for h in range(SC // 512):
                    pslice = ps[:, h * 512:(h + 1) * 512]
                    cslice = slice(h * 512, (h + 1) * 512)
                    nc.tensor.matmul(
                        pslice, lhsT=lh[:, 0:2, tsl],
                        rhs=cbt[:, 0:2, cslice],
                        start=True, stop=False, perf_mode=DR)
                    nc.tensor.matmul(
                        pslice, lhsT=lh[:, 2:4, tsl],
                        rhs=cbt[:, 2:4, cslice],
                        start=False, stop=False, perf_mode=DR)
                    nc.tensor.matmul(
                        pslice, lhsT=ones8[:],
                        rhs=cbt[:, 4:6, cslice],
                        start=False, stop=True, perf_mode=DR)
                m8s = m8buf[:, t, sc * 8:sc * 8 + 8]
                nc.vector.max(out=m8s, in_=ps[:])
                nc.vector.max_index(out=idxbuf[:, t, sc * 8:sc * 8 + 8],
                                    in_max=m8s, in_values=ps[:])

        # ---- rescue pipeline (per token tile, per stage)
        comb = {}

        def emit_a0(s, t):
            # absolute code ids (gpsimd, prefetched a round ahead)
            idxf = smallp.tile([128, NSC * 8], f32, tag="idxf")
            nc.gpsimd.tensor_copy(idxf[:], idxbuf[:, t, :])
            nc.gpsimd.tensor_add(idxf[:], idxf[:], iota_off8[:])
            comb[(s, t)] = {"idxf": idxf}

        def emit_a(s, t):
            st = comb[(s, t)]
            idxf = st["idxf"]
            g8 = smallp.tile([128, 8], f32, tag="g8")
            nc.vector.max(out=g8[:], in_=m8buf[:, t, :])
            # candidate ids via cumulative is_ge matching
            cs = smallp.tile([128, NCAND], f32, tag="cs")
            junkm = scrp.tile([128, NSC * 8], f32, tag="jm")
            for k in range(NCAND):
                nc.vector.scalar_tensor_tensor(
                    out=junkm[:], in0=m8buf[:, t, :],
                    scalar=g8[:, k:k + 1], in1=idxf[:],
                    op0=ALU.is_ge, op1=ALU.mult,
                    accum_out=cs[:, k:k + 1])
            candf = smallp.tile([128, NCAND], f32, tag="cf")
            nc.vector.tensor_copy(candf[:, 0:1], cs[:, 0:1])
            nc.vector.tensor_sub(candf[:, 1:NCAND], cs[:, 1:NCAND],
                                 cs[:, 0:NCAND - 1])
            cand32 = smallp.tile([128, NCAND], i32, tag="c32")
            nc.vector.tensor_copy(cand32[:], candf[:])
            q8 = q8p.tile([128, NCAND, PADW], f32, tag="q8")
            for k in range(NCAND):
                nc.gpsimd.indirect_dma_start(
                    out=q8[:, k, :], out_offset=None, in_=cbx_d.ap(),
                    in_offset=bass.IndirectOffsetOnAxis(
                        ap=cand32[:, k:k + 1], axis=0))
            st["candf"], st["q8"] = candf, q8

        def emit_b(s, t):
            st = comb[(s, t)]
            candf, q8 = st["candf"], st["q8"]
            sc8 = smallp.tile([128, NCAND], f32, tag="sc8")
            for k in range(NCAND):
                junk2 = scrp.tile([128, D], f32, tag="j2")
                nc.gpsimd.tensor_mul(junk2[:], q8[:, k, 0:D],
                                     z_td[:, t, :])
                nc.scalar.activation(
                    out=junk2[:], in_=junk2[:], func=AF.Copy,
                    accum_out=sc8[:, k:k + 1])
            nc.vector.tensor_add(sc8[:], sc8[:],
                                 q8[:, :, D:D + 1].squeeze())
            gm = smallp.tile([128, 1], f32, tag="gm")
            nc.vector.tensor_reduce(gm[:], sc8[:],
                                    axis=mybir.AxisListType.X,
                                    op=ALU.max)
            junk8 = smallp.tile([128, NCAND], f32, tag="j8")
            wf = smallp.tile([128, 1], f32, tag="wf")
            nc.vector.scalar_tensor_tensor(
                out=junk8[:], in0=sc8[:], scalar=gm[:],
                in1=candf[:, 0:NCAND], op0=ALU.is_ge, op1=ALU.mult,
                accum_out=wf[:])
            wi32 = smallp.tile([128, 1], i32, tag="wi")
            nc.vector.tensor_copy(wi32[:], wf[:])
            drow = drp.tile([128, D], f32, tag="dr")
            nc.gpsimd.indirect_dma_start(
                out=drow[:], out_offset=None, in_=cbD_d.ap(),
                in_offset=bass.IndirectOffsetOnAxis(ap=wi32[:], axis=0))
            st["drow"] = drow
            if s == 0:
                qrow = qtp.tile([128, D], f32, tag="qt")
                nc.gpsimd.indirect_dma_start(
                    out=qrow[:], out_offset=None,
                    in_=cbq_d.ap(),
                    in_offset=bass.IndirectOffsetOnAxis(ap=wi32[:], axis=0))
                st["qrow"] = qrow

        def emit_c(s, t):
            st = comb.pop((s, t))
            drow = st["drow"]
            if s == 0:
                nc.gpsimd.tensor_sub(z_td[:, t, :], z_td[:, t, :],
                                     st["qrow"][:])
                emit_rt8(t)
                nc.gpsimd.tensor_add(qacc[:, t, :], drow[:], db[:])
            else:
                o_t = outp.tile([128, D], f32, tag="o")
                nc.gpsimd.tensor_add(o_t[:], qacc[:, t, :], drow[:])
                nc.sync.dma_start(out_d.ap()[t * 128:(t + 1) * 128, :], o_t[:])

        def pump(ps, tiles, r, coff=6):
            n = len(tiles)
            if 0 <= r < n:
                emit_a0(ps, tiles[r])
            if 0 <= r - 1 < n:
                emit_a(ps, tiles[r - 1])
            if 0 <= r - 3 < n:
                emit_b(ps, tiles[r - 3])
            if 0 <= r - coff < n:
                emit_c(ps, tiles[r - coff])

        GROUPS = [range(0, 8), range(8, 16)]
        emit_enc_dma(0)
        prev = None
        w = 0
        for s in range(NUM_Q):
            for tset in GROUPS:
                for sc in range(NSC):
                    emit_sweep_sc(s, tset, sc)
                    if prev is not None:
                        pump(prev[0], prev[1], sc)
                    if s == 0:
                        if w == 18:
                            emit_enc_dma(1)
                        if 3 <= w <= 17 and w % 2 == 1:
                            emit_enc_tile((w - 3) // 2)
                        elif 21 <= w <= 28:
                            emit_enc_tile(w - 13)
                        w += 1
                prev = (s, list(tset))

        # tail flush: tight pump cadence, gathers pipelined at depth 2
        ps_, tiles_ = prev
        for r in range(len(tiles_) + 5):
            pump(ps_, tiles_, r, coff=5)

    nc.compile()
    return nc


_CACHE = {}


def _get_nc():
    if "nc" not in _CACHE:
        _ensure_axon_hook()
        _CACHE["nc"] = _build()
    return _CACHE["nc"]


def _host_prep(x, enc_w, enc_b, codebook, dec_w, dec_b):
    import ml_dtypes
    f8 = ml_dtypes.float8_e4m3

    x = np.asarray(x, np.float32)
    enc_w = np.asarray(enc_w, np.float32)
    enc_b = np.asarray(enc_b, np.float32)
    cb = np.ascontiguousarray(np.asarray(codebook, np.float32))
    dec_w = np.asarray(dec_w, np.float32)
    dec_b = np.asarray(dec_b, np.float32)

    flat = x.reshape(T, D)
    csq = (cb.astype(np.float64) ** 2).sum(-1).astype(np.float32)
    bias = (-0.5 * csq).astype(np.float32)

    def fp8_stream(table, bias_vec):
        """[128, NR8, K] fp8: rows 0-3 = lam*table.T k-tiles, 4-5 = bias."""
        tT = np.ascontiguousarray(table.T)                # [D, K]
        cbs = np.zeros((128, NR8, K), np.float32)
        cbs[:, :NJ, :] = (LAM * tT).reshape(NJ, 128, K).transpose(1, 0, 2)
        rem = (LAM * bias_vec).astype(np.float32).copy()
        parts = []
        for _ in range(4):
            p = np.asarray(rem, f8).astype(np.float32)
            parts.append(p)
            rem = rem - p
        cbs[0, NJ, :] = parts[0]
        cbs[1, NJ, :] = parts[1]
        cbs[0, NJ + 1, :] = parts[2]
        cbs[1, NJ + 1, :] = parts[3]
        return np.asarray(cbs, f8)

    # stage-1 sweeps on x directly: z.c == x.(cb @ enc_w) + enc_b.c
    cbE = (cb @ enc_w).astype(np.float32)
    biasE = bias + (cb @ enc_b).astype(np.float32)
    cbsE = fp8_stream(cbE, biasE)
    cbs2 = fp8_stream(cb, bias)

    # rescore table [cb row | bias | pad]; decoder-folded gather table
    cbx = np.zeros((K, PADW), np.float32)
    cbx[:, :D] = cb
    cbx[:, D] = bias
    cbD = np.ascontiguousarray(cb @ dec_w.T.astype(np.float32))
    cbq = cb

    ewT = np.ascontiguousarray(
        enc_w.T.reshape(NJ, 128, D).transpose(1, 0, 2))   # [128, NJ, D]
    ebf = np.ascontiguousarray(np.broadcast_to(enc_b, (128, D)))
    dbf = np.ascontiguousarray(np.broadcast_to(dec_b, (128, D)))
    ones8 = np.ones((128, 2, 128), f8)

    common = {"cbsE": cbsE, "cbs2": cbs2, "cbx": cbx, "cbq": cbq,
              "cbD": cbD, "ewT": ewT, "ebf": ebf, "db": dbf, "ones8": ones8}

    in_maps = []
    for sh in range(NCORES):
        shard = flat[sh * TL:(sh + 1) * TL]               # [TL, D]
        xT = np.ascontiguousarray(
            shard.T.reshape(NJ, 128, TL).transpose(1, 0, 2))
        xT8 = np.asarray(xT, f8)
        in_maps.append({"xT": xT, "xT8": xT8, **common})
    return in_maps


def _run(inputs, trace=False):
    nc = _get_nc()
    in_maps = _host_prep(**inputs)
    res = run_bass_kernel_spmd(nc, in_maps, list(range(NCORES)), trace=trace)
    outs = [res.results[s]["out"] for s in range(NCORES)]
    full = np.concatenate(outs, axis=0).reshape(B, N, D)
    return full, res


def kernel(**inputs) -> np.ndarray:
    out, _ = _run(inputs, trace=False)
    return out


def kernel_traced(**inputs):
    out, res = _run(inputs, trace=True)
    return out, res


# revision 32
# speedup vs baseline: 1.1000x; 1.0069x over previous
"""Trainium2 Bass kernel for residual-VQ autoencoder (nn_Autoencoder_45148696216751).

v3: encoder/decoder folding + DVE-minimal rescue.

Per core (data-parallel over tokens, 8 cores x 2048 tokens):
  stage-1 sweep runs directly on fp8(x) against cbE = codebook @ enc_w
  (host-folded), so the exact f32 encoder (z = x @ enc_w.T + b) overlaps
  under the sweep instead of gating it.
  2x VQ stage:
     scores[t,k] = lam*(r.c - |c|^2/2) via fp8 DoubleRow matmuls
     (4 data + 2 bias-partial DR matmuls per 1024-code superchunk)
     DVE: max8 -> m8buf (top-8 per superchunk, exact f32) and
          find_index8 -> idxbuf (within-superchunk positions)
     rescue per token tile: eps-perturbed m8 -> global max8 + find_index8
     gives the top-8 SLOTS; gpsimd gathers absolute code ids by slot;
     ONE batched indirect-DMA gather of 6 candidate rows [cb|bias];
     exact f32 rescore dots on gpsimd/scalar; winner picked by is_ge
     match; winner row gathered from cbx (residual update) and from
     cbD = codebook @ dec_w.T (host-folded decoder) for the output
     accumulation out[t] = cbD[w1] + cbD[w2] + dec_b. No decoder matmul,
     no qT transposes.
"""
import os, sys, types

os.environ.setdefault("NEURON_RT_RESET_CORES", "1")
sys.path.insert(0, '/opt/trn_rl_repo')
import numpy as np

import concourse.bass as bass
import concourse.tile as tile
from concourse import bacc, mybir
from concourse.bass_utils import run_bass_kernel_spmd
from concourse.masks import make_identity

f32 = mybir.dt.float32
fp8 = mybir.dt.float8e4
i32 = mybir.dt.int32
u16 = mybir.dt.uint16
ALU = mybir.AluOpType
DR = mybir.MatmulPerfMode.DoubleRow
AF = mybir.ActivationFunctionType

NCORES = 8
B, N, D = 4, 4096, 512
T = B * N                 # 16384 tokens
TL = T // NCORES          # 2048 tokens per core
K = 16384                 # codebook size
NT = TL // 128            # 16 token tiles per core
NJ = D // 128             # 4 contraction tiles
SC = 1024                 # superchunk (2 psum banks)
NSC = K // SC             # 16 superchunks
NR8 = 6                   # fp8 stream rows (4 cb + 2 bias-partial)
NUM_Q = 2
NCAND = 6                 # rescued candidates per token
PADW = 520                # cbx row: 512 cb + 1 bias + pad
LAM = 0.5                 # score scale so |lam*bias| fits fp8 range
QSCALE = 6.0              # pack: quantize m8 score to 1/6 grid
QOFF = 1252.0             # pack offset so quantized scores are >= 0
QMUL = 16384.0            # pack: id rides the low 14 bits


def _ensure_axon_hook():
    """Register the NTFF profile hook (missing antenv.axon_hooks shim)."""
    if "antenv.axon_hooks" in sys.modules:
        return
    mod = types.ModuleType("antenv.axon_hooks")
    _h = [None]
    mod.set_axon_ntff_profile_hook = lambda h: _h.__setitem__(0, h)
    mod.get_axon_ntff_profile_hook = lambda: _h[0]
    sys.modules["antenv.axon_hooks"] = mod
    try:
        import antenv
        antenv.axon_hooks = mod
        from trn_agent_boot.trn_boot import _ntff_profile_via_ctypes
        hook = _ntff_profile_via_ctypes('/opt/axon/libaxon_pjrt.so')
        if hook is not None:
            mod.set_axon_ntff_profile_hook(hook)
    except Exception:
        pass


def _build():
    nc = bacc.Bacc("TRN2", target_bir_lowering=False, debug=False,
                   num_devices=NCORES)

    xT_d = nc.dram_tensor("xT", [128, NJ, TL], f32, kind="ExternalInput")
    xT8_d = nc.dram_tensor("xT8", [128, NJ, TL], fp8, kind="ExternalInput")
    cbsE_d = nc.dram_tensor("cbsE", [128, NR8, K], fp8, kind="ExternalInput")
    cbs2_d = nc.dram_tensor("cbs2", [128, NR8, K], fp8, kind="ExternalInput")
    cbx_d = nc.dram_tensor("cbx", [K, PADW], f32, kind="ExternalInput")
    cbq_d = nc.dram_tensor("cbq", [K, D], f32, kind="ExternalInput")
    cbD_d = nc.dram_tensor("cbD", [K, D], f32, kind="ExternalInput")
    ewT_d = nc.dram_tensor("ewT", [128, NJ, D], f32, kind="ExternalInput")
    ebf_d = nc.dram_tensor("ebf", [128, D], f32, kind="ExternalInput")
    db_d = nc.dram_tensor("db", [128, D], f32, kind="ExternalInput")
    ones8_d = nc.dram_tensor("ones8", [128, 2, 128], fp8, kind="ExternalInput")
    out_d = nc.dram_tensor("out", [TL, D], f32, kind="ExternalOutput")

    from contextlib import ExitStack
    with tile.TileContext(nc) as tc, ExitStack() as ctx:
        big = ctx.enter_context(tc.tile_pool(name="big", bufs=1))
        xp = ctx.enter_context(tc.tile_pool(name="xp", bufs=1))
        cbp = ctx.enter_context(tc.tile_pool(name="cbp", bufs=3))
        q8p = ctx.enter_context(tc.tile_pool(name="q8p", bufs=2))
        qtp = ctx.enter_context(tc.tile_pool(name="qtp", bufs=3))
        drp = ctx.enter_context(tc.tile_pool(name="drp", bufs=3))
        outp = ctx.enter_context(tc.tile_pool(name="outp", bufs=2))
        scrp = ctx.enter_context(tc.tile_pool(name="scr", bufs=2))
        smallp = ctx.enter_context(tc.tile_pool(name="small", bufs=8))
        psc = ctx.enter_context(tc.tile_pool(name="psc", bufs=3, space="PSUM"))
        psm = ctx.enter_context(tc.tile_pool(name="psm", bufs=2, space="PSUM"))

        # ---- persistent tiles
        xT8 = big.tile([128, NJ, TL], fp8)     # stage-1 lhsT (fp8 of x)
        rT8 = big.tile([128, NJ, TL], fp8)     # stage-2 lhsT (fp8 residual)
        z_td = big.tile([128, NT, D], f32)     # exact residual, [t, d] layout
        qacc = big.tile([128, NT, D], f32)     # output accumulator
        ewT = big.tile([128, NJ, D], f32)
        ebf = big.tile([128, D], f32)
        db = big.tile([128, D], f32)
        ones8 = big.tile([128, 2, 128], fp8)
        ident = big.tile([128, 128], f32)
        m8buf = big.tile([128, NT, NSC * 8], f32)
        idxbuf = big.tile([128, NT, NSC * 8], u16)
        iota_off8 = big.tile([128, NSC * 8], f32)

        nc.sync.dma_start(xT8[:, :, 0:TL // 2], xT8_d.ap()[:, :, 0:TL // 2])
        nc.sync.dma_start(xT8[:, :, TL // 2:TL], xT8_d.ap()[:, :, TL // 2:TL])
        nc.sync.dma_start(ewT[:], ewT_d.ap())
        nc.sync.dma_start(ebf[:], ebf_d.ap())
        nc.sync.dma_start(db[:], db_d.ap())
        nc.sync.dma_start(ones8[:], ones8_d.ap())
        make_identity(nc, ident[:])
        nc.gpsimd.iota(iota_off8[:], pattern=[[SC, NSC], [0, 8]], base=0,
                       channel_multiplier=0,
                       allow_small_or_imprecise_dtypes=True)


        # ---- encoder (overlapped under stage-1 sweep): z_td = x @ ewT + eb
        HL = TL // 2
        xth = [None, None]

        def emit_enc_dma(half):
            xth[half] = xp.tile([128, NJ, HL], f32, tag="x", name="xth")
            nc.sync.dma_start(xth[half][:],
                              xT_d.ap()[:, :, half * HL:(half + 1) * HL])

        def emit_enc_tile(t):
            half, t2 = divmod(t, HL // 128)
            psz = psm.tile([128, 512], f32, tag="m")
            for j in range(NJ):
                nc.tensor.matmul(
                    psz[:], lhsT=xth[half][:, j, t2 * 128:(t2 + 1) * 128],
                    rhs=ewT[:, j, :],
                    start=(j == 0), stop=(j == NJ - 1))
            nc.scalar.copy(z_td[:, t, :], psz[:])
            nc.gpsimd.tensor_add(z_td[:, t, :], z_td[:, t, :], ebf[:])

        def emit_rt8(t):
            """rT8[:, :, t*128:(t+1)*128] = fp8(transpose(z_td[:, t, :]))."""
            tp = psm.tile([128, NJ, 128], f32, tag="m")
            for j in range(NJ):
                nc.tensor.transpose(tp[:, j, :],
                                    z_td[:, t, j * 128:(j + 1) * 128],
                                    ident[:])
            nc.scalar.copy(rT8[:, :, t * 128:(t + 1) * 128], tp[:])

        # ---- fp8 sweep + scans
        lhs = [xT8, rT8]
        cbs_d = [cbsE_d, cbs2_d]

        def emit_sweep_sc(s, tset, sc):
            cbt = cbp.tile([128, NR8, SC], fp8, tag="cbt")
            nc.sync.dma_start(cbt[:], cbs_d[s].ap()[:, :, sc * SC:(sc + 1) * SC])
            lh = lhs[s]
            for t in tset:
                tsl = slice(t * 128, (t + 1) * 128)
                ps = psc.tile([128, SC], f32, tag="sc")
                for h in range(SC // 512):
                    pslice = ps[:, h * 512:(h + 1) * 512]
                    cslice = slice(h * 512, (h + 1) * 512)
                    nc.tensor.matmul(
                        pslice, lhsT=lh[:, 0:2, tsl],
                        rhs=cbt[:, 0:2, cslice],
                        start=True, stop=False, perf_mode=DR)
                    nc.tensor.matmul(
                        pslice, lhsT=lh[:, 2:4, tsl],
                        rhs=cbt[:, 2:4, cslice],
                        start=False, stop=False, perf_mode=DR)
                    nc.tensor.matmul(
                        pslice, lhsT=ones8[:],
                        rhs=cbt[:, 4:6, cslice],
                        start=False, stop=True, perf_mode=DR)
                m8s = m8buf[:, t, sc * 8:sc * 8 + 8]
                nc.vector.max(out=m8s, in_=ps[:])
                nc.vector.max_index(out=idxbuf[:, t, sc * 8:sc * 8 + 8],
                                    in_max=m8s, in_values=ps[:])

        # ---- rescue pipeline (per token tile, per stage)
        comb = {}

        def emit_a0(s, t):
            # absolute code ids (gpsimd, prefetched a round ahead)
            idxf = smallp.tile([128, NSC * 8], f32, tag="idxf")
            nc.gpsimd.tensor_copy(idxf[:], idxbuf[:, t, :])
            nc.gpsimd.tensor_add(idxf[:], idxf[:], iota_off8[:])
            comb[(s, t)] = {"idxf": idxf}

        def emit_a(s, t):
            st = comb[(s, t)]
            idxf = st["idxf"]
            g8 = smallp.tile([128, 8], f32, tag="g8")
            nc.vector.max(out=g8[:], in_=m8buf[:, t, :])
            # candidate ids via cumulative is_ge matching
            cs = smallp.tile([128, NCAND], f32, tag="cs")
            junkm = scrp.tile([128, NSC * 8], f32, tag="jm")
            for k in range(NCAND):
                nc.vector.scalar_tensor_tensor(
                    out=junkm[:], in0=m8buf[:, t, :],
                    scalar=g8[:, k:k + 1], in1=idxf[:],
                    op0=ALU.is_ge, op1=ALU.mult,
                    accum_out=cs[:, k:k + 1])
            candf = smallp.tile([128, NCAND], f32, tag="cf")
            nc.vector.tensor_copy(candf[:, 0:1], cs[:, 0:1])
            nc.vector.tensor_sub(candf[:, 1:NCAND], cs[:, 1:NCAND],
                                 cs[:, 0:NCAND - 1])
            cand32 = smallp.tile([128, NCAND], i32, tag="c32")
            nc.vector.tensor_copy(cand32[:], candf[:])
            q8 = q8p.tile([128, NCAND, PADW], f32, tag="q8")
            for k in range(NCAND):
                nc.gpsimd.indirect_dma_start(
                    out=q8[:, k, :], out_offset=None, in_=cbx_d.ap(),
                    in_offset=bass.IndirectOffsetOnAxis(
                        ap=cand32[:, k:k + 1], axis=0))
            st["candf"], st["q8"] = candf, q8

        def emit_b(s, t):
            st = comb[(s, t)]
            candf, q8 = st["candf"], st["q8"]
            sc8 = smallp.tile([128, NCAND], f32, tag="sc8")
            for k in range(NCAND):
                junk2 = scrp.tile([128, D], f32, tag="j2")
                nc.gpsimd.tensor_mul(junk2[:], q8[:, k, 0:D],
                                     z_td[:, t, :])
                nc.scalar.activation(
                    out=junk2[:], in_=junk2[:], func=AF.Copy,
                    accum_out=sc8[:, k:k + 1])
            nc.vector.tensor_add(sc8[:], sc8[:],
                                 q8[:, :, D:D + 1].squeeze())
            gm = smallp.tile([128, 1], f32, tag="gm")
            nc.vector.tensor_reduce(gm[:], sc8[:],
                                    axis=mybir.AxisListType.X,
                                    op=ALU.max)
            junk8 = smallp.tile([128, NCAND], f32, tag="j8")
            wf = smallp.tile([128, 1], f32, tag="wf")
            nc.vector.scalar_tensor_tensor(
                out=junk8[:], in0=sc8[:], scalar=gm[:],
                in1=candf[:, 0:NCAND], op0=ALU.is_ge, op1=ALU.mult,
                accum_out=wf[:])
            wi32 = smallp.tile([128, 1], i32, tag="wi")
            nc.vector.tensor_copy(wi32[:], wf[:])
            drow = drp.tile([128, D], f32, tag="dr")
            nc.gpsimd.indirect_dma_start(
                out=drow[:], out_offset=None, in_=cbD_d.ap(),
                in_offset=bass.IndirectOffsetOnAxis(ap=wi32[:], axis=0))
            st["drow"] = drow
            if s == 0:
                qrow = qtp.tile([128, D], f32, tag="qt")
                nc.gpsimd.indirect_dma_start(
                    out=qrow[:], out_offset=None,
                    in_=cbq_d.ap(),
                    in_offset=bass.IndirectOffsetOnAxis(ap=wi32[:], axis=0))
                st["qrow"] = qrow

        def emit_c(s, t):
            st = comb.pop((s, t))
            drow = st["drow"]
            if s == 0:
                nc.gpsimd.tensor_sub(z_td[:, t, :], z_td[:, t, :],
                                     st["qrow"][:])
                emit_rt8(t)
                nc.gpsimd.tensor_add(qacc[:, t, :], drow[:], db[:])
            else:
                o_t = outp.tile([128, D], f32, tag="o")
                nc.gpsimd.tensor_add(o_t[:], qacc[:, t, :], drow[:])
                nc.sync.dma_start(out_d.ap()[t * 128:(t + 1) * 128, :], o_t[:])

        def pump(ps, tiles, r, coff=6):
            n = len(tiles)
            if 0 <= r < n:
                emit_a0(ps, tiles[r])
            if 0 <= r - 1 < n:
                emit_a(ps, tiles[r - 1])
            if 0 <= r - 3 < n:
                emit_b(ps, tiles[r - 3])
            if 0 <= r - coff < n:
                emit_c(ps, tiles[r - coff])

        GROUPS = [
            [range(0, 8), range(8, 16)],
            [range(0, 8), range(8, 12), range(12, 14), range(14, 16)],
        ]
        emit_enc_dma(0)
        prev = None
        w = 0
        for s in range(NUM_Q):
            for tset in GROUPS[s]:
                for sc in range(NSC):
                    emit_sweep_sc(s, tset, sc)
                    if prev is not None:
                        pump(prev[0], prev[1], sc)
                    if s == 0:
                        if w == 18:
                            emit_enc_dma(1)
                        if 3 <= w <= 17 and w % 2 == 1:
                            emit_enc_tile((w - 3) // 2)
                        elif 21 <= w <= 28:
                            emit_enc_tile(w - 13)
                        w += 1
                prev = (s, list(tset))

        # tail flush: tight pump cadence, gathers pipelined at depth 2
        ps_, tiles_ = prev
        for r in range(len(tiles_) + 5):
            pump(ps_, tiles_, r, coff=5)

    nc.compile()
    return nc


_CACHE = {}


def _get_nc():
    if "nc" not in _CACHE:
        _ensure_axon_hook()
        _CACHE["nc"] = _build()
    return _CACHE["nc"]


def _host_prep(x, enc_w, enc_b, codebook, dec_w, dec_b):
    import ml_dtypes
    f8 = ml_dtypes.float8_e4m3

    x = np.asarray(x, np.float32)
    enc_w = np.asarray(enc_w, np.float32)
    enc_b = np.asarray(enc_b, np.float32)
    cb = np.ascontiguousarray(np.asarray(codebook, np.float32))
    dec_w = np.asarray(dec_w, np.float32)
    dec_b = np.asarray(dec_b, np.float32)

    flat = x.reshape(T, D)
    csq = (cb.astype(np.float64) ** 2).sum(-1).astype(np.float32)
    bias = (-0.5 * csq).astype(np.float32)

    def fp8_stream(table, bias_vec):
        """[128, NR8, K] fp8: rows 0-3 = lam*table.T k-tiles, 4-5 = bias."""
        tT = np.ascontiguousarray(table.T)                # [D, K]
        cbs = np.zeros((128, NR8, K), np.float32)
        cbs[:, :NJ, :] = (LAM * tT).reshape(NJ, 128, K).transpose(1, 0, 2)
        rem = (LAM * bias_vec).astype(np.float32).copy()
        parts = []
        for _ in range(4):
            p = np.asarray(rem, f8).astype(np.float32)
            parts.append(p)
            rem = rem - p
        cbs[0, NJ, :] = parts[0]
        cbs[1, NJ, :] = parts[1]
        cbs[0, NJ + 1, :] = parts[2]
        cbs[1, NJ + 1, :] = parts[3]
        return np.asarray(cbs, f8)

    # stage-1 sweeps on x directly: z.c == x.(cb @ enc_w) + enc_b.c
    cbE = (cb @ enc_w).astype(np.float32)
    biasE = bias + (cb @ enc_b).astype(np.float32)
    cbsE = fp8_stream(cbE, biasE)
    cbs2 = fp8_stream(cb, bias)

    # rescore table [cb row | bias | pad]; decoder-folded gather table
    cbx = np.zeros((K, PADW), np.float32)
    cbx[:, :D] = cb
    cbx[:, D] = bias
    cbD = np.ascontiguousarray(cb @ dec_w.T.astype(np.float32))
    cbq = cb

    ewT = np.ascontiguousarray(
        enc_w.T.reshape(NJ, 128, D).transpose(1, 0, 2))   # [128, NJ, D]
    ebf = np.ascontiguousarray(np.broadcast_to(enc_b, (128, D)))
    dbf = np.ascontiguousarray(np.broadcast_to(dec_b, (128, D)))
    ones8 = np.ones((128, 2, 128), f8)

    common = {"cbsE": cbsE, "cbs2": cbs2, "cbx": cbx, "cbq": cbq,
              "cbD": cbD, "ewT": ewT, "ebf": ebf, "db": dbf, "ones8": ones8}

    in_maps = []
    for sh in range(NCORES):
        shard = flat[sh * TL:(sh + 1) * TL]               # [TL, D]
        xT = np.ascontiguousarray(
            shard.T.reshape(NJ, 128, TL).transpose(1, 0, 2))
        xT8 = np.asarray(xT, f8)
        in_maps.append({"xT": xT, "xT8": xT8, **common})
    return in_maps


def _run(inputs, trace=False):
    nc = _get_nc()
    in_maps = _host_prep(**inputs)
    res = run_bass_kernel_spmd(nc, in_maps, list(range(NCORES)), trace=trace)
    outs = [res.results[s]["out"] for s in range(NCORES)]
    full = np.concatenate(outs, axis=0).reshape(B, N, D)
    return full, res


def kernel(**inputs) -> np.ndarray:
    out, _ = _run(inputs, trace=False)
    return out


def kernel_traced(**inputs):
    out, res = _run(inputs, trace=True)
    return out, res


# revision 35
# speedup vs baseline: 1.1348x; 1.0316x over previous
"""Trainium2 Bass kernel for residual-VQ autoencoder (nn_Autoencoder_45148696216751).

v3: encoder/decoder folding + DVE-minimal rescue.

Per core (data-parallel over tokens, 8 cores x 2048 tokens):
  stage-1 sweep runs directly on fp8(x) against cbE = codebook @ enc_w
  (host-folded), so the exact f32 encoder (z = x @ enc_w.T + b) overlaps
  under the sweep instead of gating it.
  2x VQ stage:
     scores[t,k] = lam*(r.c - |c|^2/2) via fp8 DoubleRow matmuls
     (4 data + 2 bias-partial DR matmuls per 1024-code superchunk)
     DVE: max8 -> m8buf (top-8 per superchunk, exact f32) and
          find_index8 -> idxbuf (within-superchunk positions)
     rescue per token tile: eps-perturbed m8 -> global max8 + find_index8
     gives the top-8 SLOTS; gpsimd gathers absolute code ids by slot;
     ONE batched indirect-DMA gather of 6 candidate rows [cb|bias];
     exact f32 rescore dots on gpsimd/scalar; winner picked by is_ge
     match; winner row gathered from cbx (residual update) and from
     cbD = codebook @ dec_w.T (host-folded decoder) for the output
     accumulation out[t] = cbD[w1] + cbD[w2] + dec_b. No decoder matmul,
     no qT transposes.
"""
import os, sys, types

os.environ.setdefault("NEURON_RT_RESET_CORES", "1")
sys.path.insert(0, '/opt/trn_rl_repo')
import numpy as np

import concourse.bass as bass
import concourse.tile as tile
from concourse import bacc, mybir
from concourse.bass_utils import run_bass_kernel_spmd
from concourse.masks import make_identity

f32 = mybir.dt.float32
fp8 = mybir.dt.float8e4
i32 = mybir.dt.int32
u16 = mybir.dt.uint16
ALU = mybir.AluOpType
DR = mybir.MatmulPerfMode.DoubleRow
AF = mybir.ActivationFunctionType

NCORES = 8
B, N, D = 4, 4096, 512
T = B * N                 # 16384 tokens
TL = T // NCORES          # 2048 tokens per core
K = 16384                 # codebook size
NT = TL // 128            # 16 token tiles per core
NJ = D // 128             # 4 contraction tiles
SC = 1024                 # superchunk (2 psum banks)
NSC = K // SC             # 16 superchunks
NR8 = 6                   # fp8 stream rows (4 cb + 2 bias-partial)
NUM_Q = 2
NCAND = 6                 # rescued candidates per token
PADW = 520                # cbx row: 512 cb + 1 bias + pad
LAM = 0.5                 # score scale so |lam*bias| fits fp8 range
QSCALE = 6.0              # pack: quantize m8 score to 1/6 grid
QOFF = 1252.0             # pack offset so quantized scores are >= 0
QMUL = 16384.0            # pack: id rides the low 14 bits


def _ensure_axon_hook():
    """Register the NTFF profile hook (missing antenv.axon_hooks shim)."""
    if "antenv.axon_hooks" in sys.modules:
        return
    mod = types.ModuleType("antenv.axon_hooks")
    _h = [None]
    mod.set_axon_ntff_profile_hook = lambda h: _h.__setitem__(0, h)
    mod.get_axon_ntff_profile_hook = lambda: _h[0]
    sys.modules["antenv.axon_hooks"] = mod
    try:
        import antenv
        antenv.axon_hooks = mod
        from trn_agent_boot.trn_boot import _ntff_profile_via_ctypes
        hook = _ntff_profile_via_ctypes('/opt/axon/libaxon_pjrt.so')
        if hook is not None:
            mod.set_axon_ntff_profile_hook(hook)
    except Exception:
        pass


def _build():
    nc = bacc.Bacc("TRN2", target_bir_lowering=False, debug=False,
                   num_devices=NCORES)

    xT_d = nc.dram_tensor("xT", [128, NJ, TL], f32, kind="ExternalInput")
    xT8_d = nc.dram_tensor("xT8", [128, NJ, TL], fp8, kind="ExternalInput")
    cbsE_d = nc.dram_tensor("cbsE", [128, NR8, K], fp8, kind="ExternalInput")
    cbs2_d = nc.dram_tensor("cbs2", [128, NR8, K], fp8, kind="ExternalInput")
    cbx_d = nc.dram_tensor("cbx", [K, PADW], f32, kind="ExternalInput")
    cbq_d = nc.dram_tensor("cbq", [K, D], f32, kind="ExternalInput")
    cbD_d = nc.dram_tensor("cbD", [K, D], f32, kind="ExternalInput")
    ewT_d = nc.dram_tensor("ewT", [128, NJ, D], f32, kind="ExternalInput")
    ebf_d = nc.dram_tensor("ebf", [128, D], f32, kind="ExternalInput")
    db_d = nc.dram_tensor("db", [128, D], f32, kind="ExternalInput")
    ones8_d = nc.dram_tensor("ones8", [128, 2, 128], fp8, kind="ExternalInput")
    out_d = nc.dram_tensor("out", [TL, D], f32, kind="ExternalOutput")

    from contextlib import ExitStack
    with tile.TileContext(nc) as tc, ExitStack() as ctx:
        big = ctx.enter_context(tc.tile_pool(name="big", bufs=1))
        xp = ctx.enter_context(tc.tile_pool(name="xp", bufs=1))
        cbp = ctx.enter_context(tc.tile_pool(name="cbp", bufs=3))
        q8p = ctx.enter_context(tc.tile_pool(name="q8p", bufs=2))
        qtp = ctx.enter_context(tc.tile_pool(name="qtp", bufs=3))
        drp = ctx.enter_context(tc.tile_pool(name="drp", bufs=3))
        outp = ctx.enter_context(tc.tile_pool(name="outp", bufs=2))
        scrp = ctx.enter_context(tc.tile_pool(name="scr", bufs=2))
        smallp = ctx.enter_context(tc.tile_pool(name="small", bufs=8))
        psc = ctx.enter_context(tc.tile_pool(name="psc", bufs=3, space="PSUM"))
        psm = ctx.enter_context(tc.tile_pool(name="psm", bufs=2, space="PSUM"))

        # ---- persistent tiles
        xT8 = big.tile([128, NJ, TL], fp8)     # stage-1 lhsT (fp8 of x)
        rT8 = big.tile([128, NJ, TL], fp8)     # stage-2 lhsT (fp8 residual)
        z_td = big.tile([128, NT, D], f32)     # exact residual, [t, d] layout
        qacc = big.tile([128, NT, D], f32)     # output accumulator
        ewT = big.tile([128, NJ, D], f32)
        ebf = big.tile([128, D], f32)
        db = big.tile([128, D], f32)
        ones8 = big.tile([128, 2, 128], fp8)
        ident = big.tile([128, 128], f32)
        m8buf = big.tile([128, NT, NSC * 8], f32)
        idxbuf = big.tile([128, NT, NSC * 8], u16)
        iota_off8 = big.tile([128, NSC * 8], f32)

        nc.sync.dma_start(ones8[:], ones8_d.ap())
        nc.sync.dma_start(xT8[:, :, 0:TL // 2], xT8_d.ap()[:, :, 0:TL // 2])
        nc.sync.dma_start(xT8[:, :, TL // 2:TL], xT8_d.ap()[:, :, TL // 2:TL])
        make_identity(nc, ident[:])
        nc.gpsimd.iota(iota_off8[:], pattern=[[SC, NSC], [0, 8]], base=0,
                       channel_multiplier=0,
                       allow_small_or_imprecise_dtypes=True)

        def emit_weight_dmas():
            nc.sync.dma_start(ewT[:], ewT_d.ap())
            nc.sync.dma_start(ebf[:], ebf_d.ap())
            nc.sync.dma_start(db[:], db_d.ap())


        # ---- encoder (overlapped under stage-1 sweep): z_td = x @ ewT + eb
        HL = TL // 2
        xth = [None, None]

        def emit_enc_dma(half):
            xth[half] = xp.tile([128, NJ, HL], f32, tag="x", name="xth")
            nc.sync.dma_start(xth[half][:],
                              xT_d.ap()[:, :, half * HL:(half + 1) * HL])

        def emit_enc_tile(t):
            half, t2 = divmod(t, HL // 128)
            psz = psm.tile([128, 512], f32, tag="m")
            for j in range(NJ):
                nc.tensor.matmul(
                    psz[:], lhsT=xth[half][:, j, t2 * 128:(t2 + 1) * 128],
                    rhs=ewT[:, j, :],
                    start=(j == 0), stop=(j == NJ - 1))
            nc.scalar.copy(z_td[:, t, :], psz[:])
            nc.gpsimd.tensor_add(z_td[:, t, :], z_td[:, t, :], ebf[:])

        def emit_rt8(t):
            """rT8[:, :, t*128:(t+1)*128] = fp8(transpose(z_td[:, t, :]))."""
            tp = psm.tile([128, NJ, 128], f32, tag="m")
            for j in range(NJ):
                nc.tensor.transpose(tp[:, j, :],
                                    z_td[:, t, j * 128:(j + 1) * 128],
                                    ident[:])
            nc.scalar.copy(rT8[:, :, t * 128:(t + 1) * 128], tp[:])

        # ---- fp8 sweep + scans
        lhs = [xT8, rT8]
        cbs_d = [cbsE_d, cbs2_d]

        def emit_sweep_sc(s, tset, sc):
            cbt = cbp.tile([128, NR8, SC], fp8, tag="cbt")
            nc.sync.dma_start(cbt[:], cbs_d[s].ap()[:, :, sc * SC:(sc + 1) * SC])
            lh = lhs[s]
            for t in tset:
                tsl = slice(t * 128, (t + 1) * 128)
                ps = psc.tile([128, SC], f32, tag="sc")
                for h in range(SC // 512):
                    pslice = ps[:, h * 512:(h + 1) * 512]
                    cslice = slice(h * 512, (h + 1) * 512)
                    nc.tensor.matmul(
                        pslice, lhsT=lh[:, 0:2, tsl],
                        rhs=cbt[:, 0:2, cslice],
                        start=True, stop=False, perf_mode=DR)
                    nc.tensor.matmul(
                        pslice, lhsT=lh[:, 2:4, tsl],
                        rhs=cbt[:, 2:4, cslice],
                        start=False, stop=False, perf_mode=DR)
                    nc.tensor.matmul(
                        pslice, lhsT=ones8[:],
                        rhs=cbt[:, 4:6, cslice],
                        start=False, stop=True, perf_mode=DR)
                m8s = m8buf[:, t, sc * 8:sc * 8 + 8]
                nc.vector.max(out=m8s, in_=ps[:])
                nc.vector.max_index(out=idxbuf[:, t, sc * 8:sc * 8 + 8],
                                    in_max=m8s, in_values=ps[:])

        # ---- rescue pipeline (per token tile, per stage)
        comb = {}

        def emit_a0(s, t):
            # absolute code ids (gpsimd, prefetched a round ahead)
            idxf = smallp.tile([128, NSC * 8], f32, tag="idxf")
            nc.gpsimd.tensor_copy(idxf[:], idxbuf[:, t, :])
            nc.gpsimd.tensor_add(idxf[:], idxf[:], iota_off8[:])
            comb[(s, t)] = {"idxf": idxf}

        def emit_a(s, t):
            st = comb[(s, t)]
            idxf = st["idxf"]
            g8 = smallp.tile([128, 8], f32, tag="g8")
            nc.vector.max(out=g8[:], in_=m8buf[:, t, :])
            # candidate ids via cumulative is_ge matching
            cs = smallp.tile([128, NCAND], f32, tag="cs")
            junkm = scrp.tile([128, NSC * 8], f32, tag="jm")
            for k in range(NCAND):
                nc.vector.scalar_tensor_tensor(
                    out=junkm[:], in0=m8buf[:, t, :],
                    scalar=g8[:, k:k + 1], in1=idxf[:],
                    op0=ALU.is_ge, op1=ALU.mult,
                    accum_out=cs[:, k:k + 1])
            candf = smallp.tile([128, NCAND], f32, tag="cf")
            nc.vector.tensor_copy(candf[:, 0:1], cs[:, 0:1])
            nc.vector.tensor_sub(candf[:, 1:NCAND], cs[:, 1:NCAND],
                                 cs[:, 0:NCAND - 1])
            cand32 = smallp.tile([128, NCAND], i32, tag="c32")
            nc.vector.tensor_copy(cand32[:], candf[:])
            q8 = q8p.tile([128, NCAND, PADW], f32, tag="q8")
            for k in range(NCAND):
                nc.gpsimd.indirect_dma_start(
                    out=q8[:, k, :], out_offset=None, in_=cbx_d.ap(),
                    in_offset=bass.IndirectOffsetOnAxis(
                        ap=cand32[:, k:k + 1], axis=0))
            st["candf"], st["q8"] = candf, q8

        def emit_b(s, t, dve_dots=False):
            st = comb[(s, t)]
            candf, q8 = st["candf"], st["q8"]
            sc8 = smallp.tile([128, NCAND], f32, tag="sc8")
            if dve_dots:
                junk = scrp.tile([128, D], f32, tag="j2")
                for k in range(NCAND):
                    nc.vector.scalar_tensor_tensor(
                        out=junk[:], in0=q8[:, k, 0:D], scalar=1.0,
                        in1=z_td[:, t, :], op0=ALU.bypass, op1=ALU.mult,
                        accum_out=sc8[:, k:k + 1])
            else:
                for k in range(NCAND):
                    junk2 = scrp.tile([128, D], f32, tag="j2")
                    nc.gpsimd.tensor_mul(junk2[:], q8[:, k, 0:D],
                                         z_td[:, t, :])
                    nc.scalar.activation(
                        out=junk2[:], in_=junk2[:], func=AF.Copy,
                        accum_out=sc8[:, k:k + 1])
            nc.vector.tensor_add(sc8[:], sc8[:],
                                 q8[:, :, D:D + 1].squeeze())
            gm = smallp.tile([128, 1], f32, tag="gm")
            nc.vector.tensor_reduce(gm[:], sc8[:],
                                    axis=mybir.AxisListType.X,
                                    op=ALU.max)
            junk8 = smallp.tile([128, NCAND], f32, tag="j8")
            wf = smallp.tile([128, 1], f32, tag="wf")
            nc.vector.scalar_tensor_tensor(
                out=junk8[:], in0=sc8[:], scalar=gm[:],
                in1=candf[:, 0:NCAND], op0=ALU.is_ge, op1=ALU.mult,
                accum_out=wf[:])
            wi32 = smallp.tile([128, 1], i32, tag="wi")
            nc.vector.tensor_copy(wi32[:], wf[:])
            drow = drp.tile([128, D], f32, tag="dr")
            nc.gpsimd.indirect_dma_start(
                out=drow[:], out_offset=None, in_=cbD_d.ap(),
                in_offset=bass.IndirectOffsetOnAxis(ap=wi32[:], axis=0))
            st["drow"] = drow
            if s == 0:
                qrow = qtp.tile([128, D], f32, tag="qt")
                nc.gpsimd.indirect_dma_start(
                    out=qrow[:], out_offset=None,
                    in_=cbq_d.ap(),
                    in_offset=bass.IndirectOffsetOnAxis(ap=wi32[:], axis=0))
                st["qrow"] = qrow

        def emit_c(s, t):
            st = comb.pop((s, t))
            drow = st["drow"]
            if s == 0:
                nc.gpsimd.tensor_sub(z_td[:, t, :], z_td[:, t, :],
                                     st["qrow"][:])
                emit_rt8(t)
                nc.gpsimd.tensor_add(qacc[:, t, :], drow[:], db[:])
            else:
                o_t = outp.tile([128, D], f32, tag="o")
                nc.gpsimd.tensor_add(o_t[:], qacc[:, t, :], drow[:])
                nc.sync.dma_start(out_d.ap()[t * 128:(t + 1) * 128, :], o_t[:])

        def pump(ps, tiles, r, coff=6, dve_dots=False):
            n = len(tiles)
            if 0 <= r < n:
                emit_a0(ps, tiles[r])
            if 0 <= r - 1 < n:
                emit_a(ps, tiles[r - 1])
            if 0 <= r - 3 < n:
                emit_b(ps, tiles[r - 3], dve_dots=dve_dots)
            if 0 <= r - coff < n:
                emit_c(ps, tiles[r - coff])

        GROUPS = [
            [range(0, 8), range(8, 16)],
            [range(0, 8), range(8, 12), range(12, 14), range(14, 16)],
        ]
        # encoder tile / xT half-DMA schedule on the global window index
        ENC_SCHED = {7: 0, 9: 1, 11: 2, 13: 3, 15: 4, 17: 5, 19: 6, 21: 7,
                     25: 8, 26: 9, 27: 10, 28: 11, 29: 12, 30: 13, 31: 14,
                     32: 15}
        XT_SCHED = {1: 0, 22: 1}
        prev = None
        w = 0
        for s in range(NUM_Q):
            for gi, tset in enumerate(GROUPS[s]):
                small = len(tset) <= 4
                for sc in range(NSC):
                    emit_sweep_sc(s, tset, sc)
                    if prev is not None:
                        pump(prev[0], prev[1], sc,
                             dve_dots=(s == 1 and gi >= 2))
                    if w == 1:
                        emit_weight_dmas()
                    if w in XT_SCHED:
                        emit_enc_dma(XT_SCHED[w])
                    if w in ENC_SCHED:
                        emit_enc_tile(ENC_SCHED[w])
                    w += 1
                prev = (s, list(tset))

        # tail flush: tight pump cadence, rescore dots on the idle DVE
        ps_, tiles_ = prev
        for r in range(len(tiles_) + 5):
            pump(ps_, tiles_, r, coff=5, dve_dots=True)

    nc.compile()
    return nc


_CACHE = {}


def _get_nc():
    if "nc" not in _CACHE:
        _ensure_axon_hook()
        _CACHE["nc"] = _build()
    return _CACHE["nc"]


def _host_prep(x, enc_w, enc_b, codebook, dec_w, dec_b):
    import ml_dtypes
    f8 = ml_dtypes.float8_e4m3

    x = np.asarray(x, np.float32)
    enc_w = np.asarray(enc_w, np.float32)
    enc_b = np.asarray(enc_b, np.float32)
    cb = np.ascontiguousarray(np.asarray(codebook, np.float32))
    dec_w = np.asarray(dec_w, np.float32)
    dec_b = np.asarray(dec_b, np.float32)

    flat = x.reshape(T, D)
    csq = (cb.astype(np.float64) ** 2).sum(-1).astype(np.float32)
    bias = (-0.5 * csq).astype(np.float32)

    def fp8_stream(table, bias_vec):
        """[128, NR8, K] fp8: rows 0-3 = lam*table.T k-tiles, 4-5 = bias."""
        tT = np.ascontiguousarray(table.T)                # [D, K]
        cbs = np.zeros((128, NR8, K), np.float32)
        cbs[:, :NJ, :] = (LAM * tT).reshape(NJ, 128, K).transpose(1, 0, 2)
        rem = (LAM * bias_vec).astype(np.float32).copy()
        parts = []
        for _ in range(4):
            p = np.asarray(rem, f8).astype(np.float32)
            parts.append(p)
            rem = rem - p
        cbs[0, NJ, :] = parts[0]
        cbs[1, NJ, :] = parts[1]
        cbs[0, NJ + 1, :] = parts[2]
        cbs[1, NJ + 1, :] = parts[3]
        return np.asarray(cbs, f8)

    # stage-1 sweeps on x directly: z.c == x.(cb @ enc_w) + enc_b.c
    cbE = (cb @ enc_w).astype(np.float32)
    biasE = bias + (cb @ enc_b).astype(np.float32)
    cbsE = fp8_stream(cbE, biasE)
    cbs2 = fp8_stream(cb, bias)

    # rescore table [cb row | bias | pad]; decoder-folded gather table
    cbx = np.zeros((K, PADW), np.float32)
    cbx[:, :D] = cb
    cbx[:, D] = bias
    cbD = np.ascontiguousarray(cb @ dec_w.T.astype(np.float32))
    cbq = cb

    ewT = np.ascontiguousarray(
        enc_w.T.reshape(NJ, 128, D).transpose(1, 0, 2))   # [128, NJ, D]
    ebf = np.ascontiguousarray(np.broadcast_to(enc_b, (128, D)))
    dbf = np.ascontiguousarray(np.broadcast_to(dec_b, (128, D)))
    ones8 = np.ones((128, 2, 128), f8)

    common = {"cbsE": cbsE, "cbs2": cbs2, "cbx": cbx, "cbq": cbq,
              "cbD": cbD, "ewT": ewT, "ebf": ebf, "db": dbf, "ones8": ones8}

    in_maps = []
    for sh in range(NCORES):
        shard = flat[sh * TL:(sh + 1) * TL]               # [TL, D]
        xT = np.ascontiguousarray(
            shard.T.reshape(NJ, 128, TL).transpose(1, 0, 2))
        xT8 = np.asarray(xT, f8)
        in_maps.append({"xT": xT, "xT8": xT8, **common})
    return in_maps


def _run(inputs, trace=False):
    nc = _get_nc()
    in_maps = _host_prep(**inputs)
    res = run_bass_kernel_spmd(nc, in_maps, list(range(NCORES)), trace=trace)
    outs = [res.results[s]["out"] for s in range(NCORES)]
    full = np.concatenate(outs, axis=0).reshape(B, N, D)
    return full, res


def kernel(**inputs) -> np.ndarray:
    out, _ = _run(inputs, trace=False)
    return out


def kernel_traced(**inputs):
    out, res = _run(inputs, trace=True)
    return out, res


# revision 36
# speedup vs baseline: 1.1348x; 1.0000x over previous
"""Trainium2 Bass kernel for residual-VQ autoencoder (nn_Autoencoder_45148696216751).

v3: encoder/decoder folding + DVE-minimal rescue pipeline.

Per core (data-parallel over tokens, 8 cores x 2048 tokens):
  stage-1 sweep runs directly on fp8(x) against cbE = codebook @ enc_w
  (host-folded; enc_b folded into the bias partials), so the exact f32
  encoder (z = x @ ewT + eb) overlaps UNDER the stage-1 sweep windows.
  2x VQ stage:
     scores[t,k] = lam*(r.c - |c|^2/2) via fp8 DoubleRow matmuls
     (4 data + 2 bias-partial DR matmuls per 1024-code superchunk)
     DVE: max8 -> m8buf (top-8 per superchunk, exact f32) and
          find_index8 -> idxbuf (within-superchunk positions).
     Rescue pipeline per token tile (pumped against the next group's
     sweep; a0/a/b/c staggered): gpsimd prefetches absolute ids
     (idxbuf + superchunk offsets); DVE global max8 + NCAND cumulative
     is_ge matches extract candidate ids; per-candidate indirect-DMA
     gathers of [cb|bias] rows; exact f32 rescore dots (gpsimd mul +
     scalar accum in the steady state, DVE STT dots in the drain tail
     where DVE is idle); winner picked by is_ge match on exact scores;
     winner row gathered from cbq (residual update, stage 1) and from
     cbD = codebook @ dec_w.T (host-folded decoder) for the output
     out[t] = cbD[w1] + cbD[w2] + dec_b. No decoder matmul, no qT
     transposes, no q_sum accumulation on the PE.
  Stage-2 tail uses shrinking groups (8/4/2/2) so each group's rescue
  hides under the next group's sweep.
"""
import os, sys, types

os.environ.setdefault("NEURON_RT_RESET_CORES", "1")
sys.path.insert(0, '/opt/trn_rl_repo')
import numpy as np

import concourse.bass as bass
import concourse.tile as tile
from concourse import bacc, mybir
from concourse.bass_utils import run_bass_kernel_spmd
from concourse.masks import make_identity

f32 = mybir.dt.float32
fp8 = mybir.dt.float8e4
i32 = mybir.dt.int32
u16 = mybir.dt.uint16
ALU = mybir.AluOpType
DR = mybir.MatmulPerfMode.DoubleRow
AF = mybir.ActivationFunctionType

NCORES = 8
B, N, D = 4, 4096, 512
T = B * N                 # 16384 tokens
TL = T // NCORES          # 2048 tokens per core
K = 16384                 # codebook size
NT = TL // 128            # 16 token tiles per core
NJ = D // 128             # 4 contraction tiles
SC = 1024                 # superchunk (2 psum banks)
NSC = K // SC             # 16 superchunks
NR8 = 6                   # fp8 stream rows (4 cb + 2 bias-partial)
NUM_Q = 2
NCAND = 6                 # rescued candidates per token
PADW = 520                # cbx row: 512 cb + 1 bias + pad
LAM = 0.5                 # score scale so |lam*bias| fits fp8 range
QSCALE = 6.0              # pack: quantize m8 score to 1/6 grid
QOFF = 1252.0             # pack offset so quantized scores are >= 0
QMUL = 16384.0            # pack: id rides the low 14 bits


def _ensure_axon_hook():
    """Register the NTFF profile hook (missing antenv.axon_hooks shim)."""
    if "antenv.axon_hooks" in sys.modules:
        return
    mod = types.ModuleType("antenv.axon_hooks")
    _h = [None]
    mod.set_axon_ntff_profile_hook = lambda h: _h.__setitem__(0, h)
    mod.get_axon_ntff_profile_hook = lambda: _h[0]
    sys.modules["antenv.axon_hooks"] = mod
    try:
        import antenv
        antenv.axon_hooks = mod
        from trn_agent_boot.trn_boot import _ntff_profile_via_ctypes
        hook = _ntff_profile_via_ctypes('/opt/axon/libaxon_pjrt.so')
        if hook is not None:
            mod.set_axon_ntff_profile_hook(hook)
    except Exception:
        pass


def _build():
    nc = bacc.Bacc("TRN2", target_bir_lowering=False, debug=False,
                   num_devices=NCORES)

    xT_d = nc.dram_tensor("xT", [128, NJ, TL], f32, kind="ExternalInput")
    xT8_d = nc.dram_tensor("xT8", [128, NJ, TL], fp8, kind="ExternalInput")
    cbsE_d = nc.dram_tensor("cbsE", [128, NR8, K], fp8, kind="ExternalInput")
    cbs2_d = nc.dram_tensor("cbs2", [128, NR8, K], fp8, kind="ExternalInput")
    cbx_d = nc.dram_tensor("cbx", [K, PADW], f32, kind="ExternalInput")
    cbq_d = nc.dram_tensor("cbq", [K, D], f32, kind="ExternalInput")
    cbD_d = nc.dram_tensor("cbD", [K, D], f32, kind="ExternalInput")
    ewT_d = nc.dram_tensor("ewT", [128, NJ, D], f32, kind="ExternalInput")
    ebf_d = nc.dram_tensor("ebf", [128, D], f32, kind="ExternalInput")
    db_d = nc.dram_tensor("db", [128, D], f32, kind="ExternalInput")
    ones8_d = nc.dram_tensor("ones8", [128, 2, 128], fp8, kind="ExternalInput")
    out_d = nc.dram_tensor("out", [TL, D], f32, kind="ExternalOutput")

    from contextlib import ExitStack
    with tile.TileContext(nc) as tc, ExitStack() as ctx:
        big = ctx.enter_context(tc.tile_pool(name="big", bufs=1))
        xp = ctx.enter_context(tc.tile_pool(name="xp", bufs=1))
        cbp = ctx.enter_context(tc.tile_pool(name="cbp", bufs=3))
        q8p = ctx.enter_context(tc.tile_pool(name="q8p", bufs=2))
        qtp = ctx.enter_context(tc.tile_pool(name="qtp", bufs=3))
        drp = ctx.enter_context(tc.tile_pool(name="drp", bufs=3))
        outp = ctx.enter_context(tc.tile_pool(name="outp", bufs=2))
        scrp = ctx.enter_context(tc.tile_pool(name="scr", bufs=2))
        smallp = ctx.enter_context(tc.tile_pool(name="small", bufs=8))
        psc = ctx.enter_context(tc.tile_pool(name="psc", bufs=3, space="PSUM"))
        psm = ctx.enter_context(tc.tile_pool(name="psm", bufs=2, space="PSUM"))

        # ---- persistent tiles
        xT8 = big.tile([128, NJ, TL], fp8)     # stage-1 lhsT (fp8 of x)
        rT8 = big.tile([128, NJ, TL], fp8)     # stage-2 lhsT (fp8 residual)
        z_td = big.tile([128, NT, D], f32)     # exact residual, [t, d] layout
        qacc = big.tile([128, NT, D], f32)     # output accumulator
        ewT = big.tile([128, NJ, D], f32)
        ebf = big.tile([128, D], f32)
        db = big.tile([128, D], f32)
        ones8 = big.tile([128, 2, 128], fp8)
        ident = big.tile([128, 128], f32)
        m8buf = big.tile([128, NT, NSC * 8], f32)
        idxbuf = big.tile([128, NT, NSC * 8], u16)
        iota_off8 = big.tile([128, NSC * 8], f32)

        nc.sync.dma_start(ones8[:], ones8_d.ap())
        nc.sync.dma_start(xT8[:, :, 0:TL // 2], xT8_d.ap()[:, :, 0:TL // 2])
        nc.sync.dma_start(xT8[:, :, TL // 2:TL], xT8_d.ap()[:, :, TL // 2:TL])
        make_identity(nc, ident[:])
        nc.gpsimd.iota(iota_off8[:], pattern=[[SC, NSC], [0, 8]], base=0,
                       channel_multiplier=0,
                       allow_small_or_imprecise_dtypes=True)

        def emit_weight_dmas():
            nc.sync.dma_start(ewT[:], ewT_d.ap())
            nc.sync.dma_start(ebf[:], ebf_d.ap())
            nc.sync.dma_start(db[:], db_d.ap())


        # ---- encoder (overlapped under stage-1 sweep): z_td = x @ ewT + eb
        HL = TL // 2
        xth = [None, None]

        def emit_enc_dma(half):
            xth[half] = xp.tile([128, NJ, HL], f32, tag="x", name="xth")
            nc.sync.dma_start(xth[half][:],
                              xT_d.ap()[:, :, half * HL:(half + 1) * HL])

        def emit_enc_tile(t):
            half, t2 = divmod(t, HL // 128)
            psz = psm.tile([128, 512], f32, tag="m")
            for j in range(NJ):
                nc.tensor.matmul(
                    psz[:], lhsT=xth[half][:, j, t2 * 128:(t2 + 1) * 128],
                    rhs=ewT[:, j, :],
                    start=(j == 0), stop=(j == NJ - 1))
            nc.scalar.copy(z_td[:, t, :], psz[:])
            nc.gpsimd.tensor_add(z_td[:, t, :], z_td[:, t, :], ebf[:])

        def emit_rt8(t):
            """rT8[:, :, t*128:(t+1)*128] = fp8(transpose(z_td[:, t, :]))."""
            tp = psm.tile([128, NJ, 128], f32, tag="m")
            for j in range(NJ):
                nc.tensor.transpose(tp[:, j, :],
                                    z_td[:, t, j * 128:(j + 1) * 128],
                                    ident[:])
            nc.scalar.copy(rT8[:, :, t * 128:(t + 1) * 128], tp[:])

        # ---- fp8 sweep + scans
        lhs = [xT8, rT8]
        cbs_d = [cbsE_d, cbs2_d]

        def emit_sweep_sc(s, tset, sc):
            cbt = cbp.tile([128, NR8, SC], fp8, tag="cbt")
            nc.sync.dma_start(cbt[:], cbs_d[s].ap()[:, :, sc * SC:(sc + 1) * SC])
            lh = lhs[s]
            for t in tset:
                tsl = slice(t * 128, (t + 1) * 128)
                ps = psc.tile([128, SC], f32, tag="sc")
                for h in range(SC // 512):
                    pslice = ps[:, h * 512:(h + 1) * 512]
                    cslice = slice(h * 512, (h + 1) * 512)
                    nc.tensor.matmul(
                        pslice, lhsT=lh[:, 0:2, tsl],
                        rhs=cbt[:, 0:2, cslice],
                        start=True, stop=False, perf_mode=DR)
                    nc.tensor.matmul(
                        pslice, lhsT=lh[:, 2:4, tsl],
                        rhs=cbt[:, 2:4, cslice],
                        start=False, stop=False, perf_mode=DR)
                    nc.tensor.matmul(
                        pslice, lhsT=ones8[:],
                        rhs=cbt[:, 4:6, cslice],
                        start=False, stop=True, perf_mode=DR)
                m8s = m8buf[:, t, sc * 8:sc * 8 + 8]
                nc.vector.max(out=m8s, in_=ps[:])
                nc.vector.max_index(out=idxbuf[:, t, sc * 8:sc * 8 + 8],
                                    in_max=m8s, in_values=ps[:])

        # ---- rescue pipeline (per token tile, per stage)
        comb = {}

        def emit_a0(s, t):
            # absolute code ids (gpsimd, prefetched a round ahead)
            idxf = smallp.tile([128, NSC * 8], f32, tag="idxf")
            nc.gpsimd.tensor_copy(idxf[:], idxbuf[:, t, :])
            nc.gpsimd.tensor_add(idxf[:], idxf[:], iota_off8[:])
            comb[(s, t)] = {"idxf": idxf}

        def emit_a(s, t):
            st = comb[(s, t)]
            idxf = st["idxf"]
            g8 = smallp.tile([128, 8], f32, tag="g8")
            nc.vector.max(out=g8[:], in_=m8buf[:, t, :])
            # candidate ids via cumulative is_ge matching
            cs = smallp.tile([128, NCAND], f32, tag="cs")
            junkm = scrp.tile([128, NSC * 8], f32, tag="jm")
            for k in range(NCAND):
                nc.vector.scalar_tensor_tensor(
                    out=junkm[:], in0=m8buf[:, t, :],
                    scalar=g8[:, k:k + 1], in1=idxf[:],
                    op0=ALU.is_ge, op1=ALU.mult,
                    accum_out=cs[:, k:k + 1])
            candf = smallp.tile([128, NCAND], f32, tag="cf")
            nc.vector.tensor_copy(candf[:, 0:1], cs[:, 0:1])
            nc.vector.tensor_sub(candf[:, 1:NCAND], cs[:, 1:NCAND],
                                 cs[:, 0:NCAND - 1])
            cand32 = smallp.tile([128, NCAND], i32, tag="c32")
            nc.vector.tensor_copy(cand32[:], candf[:])
            q8 = q8p.tile([128, NCAND, PADW], f32, tag="q8")
            for k in range(NCAND):
                nc.gpsimd.indirect_dma_start(
                    out=q8[:, k, :], out_offset=None, in_=cbx_d.ap(),
                    in_offset=bass.IndirectOffsetOnAxis(
                        ap=cand32[:, k:k + 1], axis=0))
            st["candf"], st["q8"] = candf, q8

        def emit_b(s, t, dve_dots=False):
            st = comb[(s, t)]
            candf, q8 = st["candf"], st["q8"]
            sc8 = smallp.tile([128, NCAND], f32, tag="sc8")
            if dve_dots:
                junk = scrp.tile([128, D], f32, tag="j2")
                for k in range(NCAND):
                    nc.vector.scalar_tensor_tensor(
                        out=junk[:], in0=q8[:, k, 0:D], scalar=1.0,
                        in1=z_td[:, t, :], op0=ALU.bypass, op1=ALU.mult,
                        accum_out=sc8[:, k:k + 1])
            else:
                for k in range(NCAND):
                    junk2 = scrp.tile([128, D], f32, tag="j2")
                    nc.gpsimd.tensor_mul(junk2[:], q8[:, k, 0:D],
                                         z_td[:, t, :])
                    nc.scalar.activation(
                        out=junk2[:], in_=junk2[:], func=AF.Copy,
                        accum_out=sc8[:, k:k + 1])
            nc.vector.tensor_add(sc8[:], sc8[:],
                                 q8[:, :, D:D + 1].squeeze())
            gm = smallp.tile([128, 1], f32, tag="gm")
            nc.vector.tensor_reduce(gm[:], sc8[:],
                                    axis=mybir.AxisListType.X,
                                    op=ALU.max)
            junk8 = smallp.tile([128, NCAND], f32, tag="j8")
            wf = smallp.tile([128, 1], f32, tag="wf")
            nc.vector.scalar_tensor_tensor(
                out=junk8[:], in0=sc8[:], scalar=gm[:],
                in1=candf[:, 0:NCAND], op0=ALU.is_ge, op1=ALU.mult,
                accum_out=wf[:])
            wi32 = smallp.tile([128, 1], i32, tag="wi")
            nc.vector.tensor_copy(wi32[:], wf[:])
            drow = drp.tile([128, D], f32, tag="dr")
            nc.gpsimd.indirect_dma_start(
                out=drow[:], out_offset=None, in_=cbD_d.ap(),
                in_offset=bass.IndirectOffsetOnAxis(ap=wi32[:], axis=0))
            st["drow"] = drow
            if s == 0:
                qrow = qtp.tile([128, D], f32, tag="qt")
                nc.gpsimd.indirect_dma_start(
                    out=qrow[:], out_offset=None,
                    in_=cbq_d.ap(),
                    in_offset=bass.IndirectOffsetOnAxis(ap=wi32[:], axis=0))
                st["qrow"] = qrow

        def emit_c(s, t):
            st = comb.pop((s, t))
            drow = st["drow"]
            if s == 0:
                nc.gpsimd.tensor_sub(z_td[:, t, :], z_td[:, t, :],
                                     st["qrow"][:])
                emit_rt8(t)
                nc.gpsimd.tensor_add(qacc[:, t, :], drow[:], db[:])
            else:
                o_t = outp.tile([128, D], f32, tag="o")
                nc.gpsimd.tensor_add(o_t[:], qacc[:, t, :], drow[:])
                nc.sync.dma_start(out_d.ap()[t * 128:(t + 1) * 128, :], o_t[:])

        def pump(ps, tiles, r, coff=6, dve_dots=False):
            n = len(tiles)
            if 0 <= r < n:
                emit_a0(ps, tiles[r])
            if 0 <= r - 1 < n:
                emit_a(ps, tiles[r - 1])
            if 0 <= r - 3 < n:
                emit_b(ps, tiles[r - 3], dve_dots=dve_dots)
            if 0 <= r - coff < n:
                emit_c(ps, tiles[r - coff])

        GROUPS = [
            [range(0, 8), range(8, 16)],
            [range(0, 8), range(8, 12), range(12, 14), range(14, 16)],
        ]
        # encoder tile / xT half-DMA schedule on the global window index
        ENC_SCHED = {7: 0, 9: 1, 11: 2, 13: 3, 15: 4, 17: 5, 19: 6, 21: 7,
                     25: 8, 26: 9, 27: 10, 28: 11, 29: 12, 30: 13, 31: 14,
                     32: 15}
        XT_SCHED = {1: 0, 22: 1}
        prev = None
        w = 0
        for s in range(NUM_Q):
            for gi, tset in enumerate(GROUPS[s]):
                small = len(tset) <= 4
                for sc in range(NSC):
                    emit_sweep_sc(s, tset, sc)
                    if prev is not None:
                        pump(prev[0], prev[1], sc,
                             dve_dots=(s == 1 and gi >= 2))
                    if w == 1:
                        emit_weight_dmas()
                    if w in XT_SCHED:
                        emit_enc_dma(XT_SCHED[w])
                    if w in ENC_SCHED:
                        emit_enc_tile(ENC_SCHED[w])
                    w += 1
                prev = (s, list(tset))

        # tail flush: tight pump cadence, rescore dots on the idle DVE
        ps_, tiles_ = prev
        for r in range(len(tiles_) + 5):
            pump(ps_, tiles_, r, coff=5, dve_dots=True)

    nc.compile()
    return nc


_CACHE = {}


def _get_nc():
    if "nc" not in _CACHE:
        _ensure_axon_hook()
        _CACHE["nc"] = _build()
    return _CACHE["nc"]


def _host_prep(x, enc_w, enc_b, codebook, dec_w, dec_b):
    import ml_dtypes
    f8 = ml_dtypes.float8_e4m3

    x = np.asarray(x, np.float32)
    enc_w = np.asarray(enc_w, np.float32)
    enc_b = np.asarray(enc_b, np.float32)
    cb = np.ascontiguousarray(np.asarray(codebook, np.float32))
    dec_w = np.asarray(dec_w, np.float32)
    dec_b = np.asarray(dec_b, np.float32)

    flat = x.reshape(T, D)
    csq = (cb.astype(np.float64) ** 2).sum(-1).astype(np.float32)
    bias = (-0.5 * csq).astype(np.float32)

    def fp8_stream(table, bias_vec):
        """[128, NR8, K] fp8: rows 0-3 = lam*table.T k-tiles, 4-5 = bias."""
        tT = np.ascontiguousarray(table.T)                # [D, K]
        cbs = np.zeros((128, NR8, K), np.float32)
        cbs[:, :NJ, :] = (LAM * tT).reshape(NJ, 128, K).transpose(1, 0, 2)
        rem = (LAM * bias_vec).astype(np.float32).copy()
        parts = []
        for _ in range(4):
            p = np.asarray(rem, f8).astype(np.float32)
            parts.append(p)
            rem = rem - p
        cbs[0, NJ, :] = parts[0]
        cbs[1, NJ, :] = parts[1]
        cbs[0, NJ + 1, :] = parts[2]
        cbs[1, NJ + 1, :] = parts[3]
        return np.asarray(cbs, f8)

    # stage-1 sweeps on x directly: z.c == x.(cb @ enc_w) + enc_b.c
    cbE = (cb @ enc_w).astype(np.float32)
    biasE = bias + (cb @ enc_b).astype(np.float32)
    cbsE = fp8_stream(cbE, biasE)
    cbs2 = fp8_stream(cb, bias)

    # rescore table [cb row | bias | pad]; decoder-folded gather table
    cbx = np.zeros((K, PADW), np.float32)
    cbx[:, :D] = cb
    cbx[:, D] = bias
    cbD = np.ascontiguousarray(cb @ dec_w.T.astype(np.float32))
    cbq = cb

    ewT = np.ascontiguousarray(
        enc_w.T.reshape(NJ, 128, D).transpose(1, 0, 2))   # [128, NJ, D]
    ebf = np.ascontiguousarray(np.broadcast_to(enc_b, (128, D)))
    dbf = np.ascontiguousarray(np.broadcast_to(dec_b, (128, D)))
    ones8 = np.ones((128, 2, 128), f8)

    common = {"cbsE": cbsE, "cbs2": cbs2, "cbx": cbx, "cbq": cbq,
              "cbD": cbD, "ewT": ewT, "ebf": ebf, "db": dbf, "ones8": ones8}

    in_maps = []
    for sh in range(NCORES):
        shard = flat[sh * TL:(sh + 1) * TL]               # [TL, D]
        xT = np.ascontiguousarray(
            shard.T.reshape(NJ, 128, TL).transpose(1, 0, 2))
        xT8 = np.asarray(xT, f8)
        in_maps.append({"xT": xT, "xT8": xT8, **common})
    return in_maps


def _run(inputs, trace=False):
    nc = _get_nc()
    in_maps = _host_prep(**inputs)
    res = run_bass_kernel_spmd(nc, in_maps, list(range(NCORES)), trace=trace)
    outs = [res.results[s]["out"] for s in range(NCORES)]
    full = np.concatenate(outs, axis=0).reshape(B, N, D)
    return full, res


def kernel(**inputs) -> np.ndarray:
    out, _ = _run(inputs, trace=False)
    return out


def kernel_traced(**inputs):
    out, res = _run(inputs, trace=True)
    return out, res
